# revision 1
# baseline (speedup 1.0000x reference)
"""GRU-ODE Trainium2 kernel: data-parallel over 8 NeuronCores (16 samples each).

Phases per core:
  1. GRU encoder: 512 sequential steps, hidden state folded as [128, 32]
     (col = half*16 + sample). All gate matmuls accumulate in PSUM.
  2. Adaptive Dormand-Prince ODE solve: 32 intervals x up-to-16 RK steps with
     exact-identity early exit (device-side branch when all 16 samples done).
     Softplus = Ln(exp(u)+1), tanh = 2*sigmoid(2v)-1 synthesized from the
     natural_log_exp activation-table set (softplus has no HW table); the
     sigmoid affine is folded into the RK linear combinations.
  3. Readout z @ ro_w.T + ro_b on device, transposed to [16, 33*64].
"""
import sys
import numpy as np

sys.path.insert(0, "/root/.axon_site/_ro/trn_rl_repo")

import concourse.bass as bass
import concourse.bacc as bacc
import concourse.tile as tile
import concourse.mybir as mybir
from contextlib import ExitStack
from concourse.bass import ds
from concourse.bass_utils import run_bass_kernel_spmd

F32 = mybir.dt.float32
AF = mybir.ActivationFunctionType
OP = mybir.AluOpType

B, TIN, NF = 128, 512, 33
CIN, H, COUT, WIDTH = 64, 256, 64, 128
MAX_STEPS = 16
RTOL, ATOL = 1e-3, 1e-6
NCORES = 8
BL = B // NCORES  # 16 samples per core

# Dormand-Prince 5(4) tableau
A_TAB = {
    2: [0.2],
    3: [3 / 40, 9 / 40],
    4: [44 / 45, -56 / 15, 32 / 9],
    5: [19372 / 6561, -25360 / 2187, 64448 / 6561, -212 / 729],
    6: [9017 / 3168, -355 / 33, 46732 / 5247, 49 / 176, -5103 / 18656],
}
B5_TAB = {1: 35 / 384, 3: 500 / 1113, 4: 125 / 192, 5: -2187 / 6784, 6: 11 / 84}
E_TAB = {1: 71 / 57600, 3: -71 / 16695, 4: 71 / 1920,
         5: -17253 / 339200, 6: 22 / 525, 7: -1 / 40}
SUM_A = {s: float(sum(A_TAB[s])) for s in A_TAB}
SUM_B5 = float(sum(B5_TAB.values()))
SUM_E = float(sum(E_TAB.values()))
# scaled-identity slots: 0 = I, 1..5 = B5 coeffs (j=1,3,4,5,6), 6..11 = E coeffs
SID_B5 = {j: i + 1 for i, j in enumerate([1, 3, 4, 5, 6])}
SID_E = {j: i + 6 for i, j in enumerate([1, 3, 4, 5, 6, 7])}
NSID = 12


def _prep_weights(inp):
    """Host-side: transform weights into the SBUF layouts the kernel wants."""
    f = lambda a: np.ascontiguousarray(a, dtype=np.float32)
    wih, whh = np.asarray(inp["gru_wih"]), np.asarray(inp["gru_whh"])
    gb, bn = np.asarray(inp["gru_b"]), np.asarray(inp["gru_bn"])
    w0, b0 = np.asarray(inp["w0"]), np.asarray(inp["b0"])
    w1, b1 = np.asarray(inp["w1"]), np.asarray(inp["b1"])
    w2, b2 = np.asarray(inp["w2"]), np.asarray(inp["b2"])
    row, rob = np.asarray(inp["ro_w"]), np.asarray(inp["ro_b"])
    t = np.asarray(inp["t"])

    sid = np.zeros((128, NSID * 128), np.float32)
    eye = np.eye(128, dtype=np.float32)
    sid[:, 0:128] = eye
    for j, slot in SID_B5.items():
        sid[:, slot * 128:(slot + 1) * 128] = eye * np.float32(B5_TAB[j])
    for j, slot in SID_E.items():
        sid[:, slot * 128:(slot + 1) * 128] = eye * np.float32(E_TAB[j])

    w0T = w0.T  # [256, 128]
    w2T = w2.T  # [128, 256]
    roT = row.T  # [256, 64]
    return {
        "wihT": f(np.concatenate([wih.T, gb[None, :]], axis=0)),  # [65, 768]
        "whhT0": f(whh.T[:128]), "whhT1": f(whh.T[128:]),  # [128, 768] each
        "bnb": f(bn.reshape(2, 128).T),  # [128, 2]
        "w0T": f(np.concatenate([w0T[:128], w0T[128:]], axis=1)),  # [128, 256]
        "w1T": f(w1.T),  # [128, 128]
        "w2T": f(w2T),  # [128, 256]
        "b0c": f(b0[:, None]), "b1c": f(b1[:, None]),  # [128, 1]
        "b2r": f(b2[None, :]),  # [1, 256]
        "w0o": f(w0.sum(axis=1)[None, :]),  # [1, 128]
        "roT": f(np.concatenate([roT[:128], roT[128:]], axis=1)),  # [128, 128]
        "rob": f(rob[None, :]),  # [1, 64]
        "sid": f(sid),  # [128, NSID*128]
        "tf": f(t[TIN:][None, :]),  # [1, NF]
    }


def _prep_core_x(y_past, core):
    """y_past [B, TIN, CIN] -> xT_aug [65, TIN*16] for one core, col = t*16+b."""
    yc = np.asarray(y_past, np.float32)[core * BL:(core + 1) * BL]  # [16,T,64]
    xt = yc.transpose(2, 1, 0).reshape(CIN, -1)  # [64, T*16]
    return np.ascontiguousarray(
        np.concatenate([xt, np.ones((1, xt.shape[1]), np.float32)], axis=0))


def build_program(tin=TIN, nf=NF, max_steps=MAX_STEPS, check_every=True):
    nc = bacc.Bacc("TRN2", target_bir_lowering=False, debug=False)
    d = {}
    d["xT"] = nc.dram_tensor("xT", [CIN + 1, tin * BL], F32, kind="ExternalInput")
    d["tf"] = nc.dram_tensor("tf", [1, nf], F32, kind="ExternalInput")
    for nm, shp in [("wihT", [65, 768]), ("whhT0", [128, 768]), ("whhT1", [128, 768]),
                    ("bnb", [128, 2]), ("w0T", [128, 256]), ("w1T", [128, 128]),
                    ("w2T", [128, 256]), ("b0c", [128, 1]), ("b1c", [128, 1]),
                    ("b2r", [1, 256]), ("w0o", [1, 128]), ("roT", [128, 128]),
                    ("rob", [1, 64]), ("sid", [128, NSID * 128])]:
        d[nm] = nc.dram_tensor(nm, shp, F32, kind="ExternalInput")
    out_d = nc.dram_tensor("out", [BL, nf * COUT], F32, kind="ExternalOutput")

    ctx = ExitStack()
    tc = ctx.enter_context(tile.TileContext(nc))
    wp = ctx.enter_context(tc.tile_pool(name="w", bufs=1))
    sp = ctx.enter_context(tc.tile_pool(name="s", bufs=1))

    # ---- load weights & inputs ----
    sb = {}
    for nm in ["wihT", "whhT0", "whhT1", "bnb", "w0T", "w1T", "w2T", "b0c",
               "b1c", "b2r", "w0o", "roT", "rob", "sid", "tf"]:
        sb[nm] = wp.tile(list(d[nm].shape), F32, tag=nm, name=nm)
        nc.sync.dma_start(sb[nm][:], d[nm][:])
    xT = wp.tile([CIN + 1, tin * BL], F32, tag="xT")
    nchunk = 4
    cw = tin * BL // nchunk
    for k in range(nchunk):
        nc.sync.dma_start(xT[:, k * cw:(k + 1) * cw], d["xT"][:, k * cw:(k + 1) * cw])

    ones16 = wp.tile([1, BL], F32, tag="ones16")
    onesr = wp.tile([1, 128], F32, tag="onesr")
    onesc = wp.tile([128, 1], F32, tag="onesc")
    eps24 = wp.tile([1, 1], F32, tag="eps24", name="eps24")
    nc.vector.memset(eps24[:], 1e-24)
    nc.vector.memset(ones16[:], 1.0)
    nc.vector.memset(onesr[:], 1.0)
    nc.vector.memset(onesc[:], 1.0)

    # ---- state tiles (fixed addresses; live across dynamic control flow) ----
    z = sp.tile([128, 2 * BL], F32, tag="z")          # folded [hidden-half | sample]
    t_st = sp.tile([1, BL], F32, tag="t_st")
    dt_st = sp.tile([1, BL], F32, tag="dt_st")
    zsave = sp.tile([128, nf * 2 * BL], F32, tag="zsave")
    ys_sb = sp.tile([BL, nf * COUT], F32, tag="ys")

    # ================= GRU phase =================
    W2 = 2 * BL
    with tc.tile_pool(name="pg", bufs=1, space="PSUM") as pg, \
         tc.tile_pool(name="gs", bufs=1) as gs:
        Gr = pg.tile([128, W2], F32, tag="Gr")
        Gz = pg.tile([128, W2], F32, tag="Gz")
        Phn = pg.tile([128, W2], F32, tag="Phn")
        Pinn = pg.tile([128, W2], F32, tag="Pinn")
        r_sb = gs.tile([128, W2], F32, tag="r_sb")
        z_sb = gs.tile([128, W2], F32, tag="z_sb")
        q3 = gs.tile([128, W2], F32, tag="q3")
        n_sb = gs.tile([128, W2], F32, tag="n_sb")
        omz = gs.tile([128, W2], F32, tag="omz")
        zh = gs.tile([128, W2], F32, tag="zh")
        nz = gs.tile([128, W2], F32, tag="nz")
        nc.vector.memset(z[:], 0.0)

        for t in range(tin):
            xs = xT[:, t * BL:(t + 1) * BL]
            for half in (0, 1):  # r gate: whh + wih + bias all into Gr
                col = 0 * 256 + half * 128
                o = Gr[:, half * BL:(half + 1) * BL]
                nc.tensor.matmul(o, sb["whhT0"][:, col:col + 128], z[:, 0:BL], start=True, stop=False)
                nc.tensor.matmul(o, sb["whhT1"][:, col:col + 128], z[:, BL:W2], start=False, stop=False)
                nc.tensor.matmul(o, sb["wihT"][:, col:col + 128], xs, start=False, stop=True)
            for half in (0, 1):  # n gate: hn (whh only) and inn (wih only)
                col = 2 * 256 + half * 128
                o = Phn[:, half * BL:(half + 1) * BL]
                nc.tensor.matmul(o, sb["whhT0"][:, col:col + 128], z[:, 0:BL], start=True, stop=False)
                nc.tensor.matmul(o, sb["whhT1"][:, col:col + 128], z[:, BL:W2], start=False, stop=True)
                oi = Pinn[:, half * BL:(half + 1) * BL]
                nc.tensor.matmul(oi, sb["wihT"][:, col:col + 128], xs, start=True, stop=True)
            for half in (0, 1):  # z gate
                col = 1 * 256 + half * 128
                o = Gz[:, half * BL:(half + 1) * BL]
                nc.tensor.matmul(o, sb["whhT0"][:, col:col + 128], z[:, 0:BL], start=True, stop=False)
                nc.tensor.matmul(o, sb["whhT1"][:, col:col + 128], z[:, BL:W2], start=False, stop=False)
                nc.tensor.matmul(o, sb["wihT"][:, col:col + 128], xs, start=False, stop=True)

            nc.scalar.activation(r_sb[:], Gr[:], AF.Sigmoid)
            for half in (0, 1):  # (hn + bn) * r  per half (per-partition bn)
                nc.vector.scalar_tensor_tensor(
                    q3[:, half * BL:(half + 1) * BL],
                    Phn[:, half * BL:(half + 1) * BL],
                    sb["bnb"][:, half:half + 1],
                    r_sb[:, half * BL:(half + 1) * BL], OP.add, OP.mult)
            nc.vector.tensor_tensor(q3[:], q3[:], Pinn[:], OP.add)
            nc.scalar.activation(n_sb[:], q3[:], AF.Tanh)
            nc.scalar.activation(z_sb[:], Gz[:], AF.Sigmoid)
            nc.gpsimd.tensor_scalar(omz[:], z_sb[:], -1.0, 1.0, OP.mult, OP.add)
            nc.gpsimd.tensor_tensor(zh[:], z_sb[:], z[:], OP.mult)
            nc.vector.tensor_tensor(nz[:], n_sb[:], omz[:], OP.mult)
            nc.vector.tensor_tensor(z[:], nz[:], zh[:], OP.add)

    nc.vector.tensor_copy(zsave[:, 0:W2], z[:])

    # ================= ODE phase =================
    # All activations below use only Exp/Ln -> single table set.
    with tc.tile_pool(name="po", bufs=1, space="PSUM") as po, \
         tc.tile_pool(name="os", bufs=1) as osb:
        P0 = po.tile([128, W2], F32, tag="P0")   # dtb2p / y5p / acceptbp
        P1 = po.tile([128, W2], F32, tag="P1")   # odtp / errp
        P2 = po.tile([128, W2], F32, tag="P2")   # ubp / u7p / msqp
        P3 = po.tile([128, BL], F32, tag="P3")   # h1p
        P4 = po.tile([128, W2], F32, tag="P4")   # fp
        P5 = po.tile([128, BL], F32, tag="P5")   # mkp

        e0 = osb.tile([128, BL], F32, tag="e0")
        h0 = osb.tile([128, BL], F32, tag="h0")
        e1 = osb.tile([128, BL], F32, tag="e1")
        h1 = osb.tile([128, BL], F32, tag="h1")
        et = osb.tile([128, W2], F32, tag="et")
        dd = osb.tile([128, W2], F32, tag="dd")
        kk = [osb.tile([128, W2], F32, tag=f"kk{j}", name=f"kk{j}") for j in range(8)]  # 1..7 used
        acc = {s: osb.tile([128, BL], F32, tag=f"acc{s}", name=f"acc{s}") for s in range(2, 7)}
        ub = osb.tile([128, BL], F32, tag="ub")
        dtb2 = osb.tile([128, W2], F32, tag="dtb2")
        y5sb = osb.tile([128, W2], F32, tag="y5sb")
        scm = osb.tile([128, W2], F32, tag="scm")
        qt = osb.tile([128, W2], F32, tag="qt")
        q2 = osb.tile([128, W2], F32, tag="q2")
        L16 = lambda tg: osb.tile([1, BL], F32, tag=tg, name=tg)
        rem, mx, dt_use = L16("rem"), L16("mx"), L16("dt_use")
        nd, done = L16("nd"), L16("done")
        dt2 = osb.tile([1, W2], F32, tag="dt2")
        sy = osb.tile([1, W2], F32, tag="sy")
        se = osb.tile([1, W2], F32, tag="se")
        tm, lnm, f0 = L16("tm"), L16("lnm"), L16("f0")
        msq32 = osb.tile([1, W2], F32, tag="msq32", name="msq32")
        rcd = osb.tile([128, W2], F32, tag="rcd", name="rcd")
        rsc = osb.tile([128, W2], F32, tag="rsc", name="rsc")
        mx1 = osb.tile([128, W2], F32, tag="mx1", name="mx1")
        mx2 = osb.tile([128, W2], F32, tag="mx2", name="mx2")
        accI = osb.tile([128, W2], mybir.dt.int32, tag="accI", name="accI")
        doneI = osb.tile([1, BL], mybir.dt.int32, tag="doneI", name="doneI")
        fac, fac2, le, acc16 = L16("fac"), L16("fac2"), L16("le"), L16("acc16")
        acc32 = osb.tile([1, W2], F32, tag="acc32")
        st_t, cand, cand2 = L16("st_t"), L16("cand"), L16("cand2")
        remn, ndn = L16("remn"), L16("ndn")
        flag = osb.tile([1, 1], F32, tag="flag")
        flagi = [osb.tile([1, 1], mybir.dt.int32, tag=f"flagi{s}", name=f"flagi{s}")
                 for s in range(MAX_STEPS)]
        tmpF, tmpL = L16("tmpF"), L16("tmpL")

        tf_sb = sb["tf"]
        # dt0 = (tf[-1] - tf[0]) * 0.01
        nc.vector.tensor_scalar(tmpF[:], ones16[:], tf_sb[0:1, 0:1], None, OP.mult)
        nc.vector.scalar_tensor_tensor(tmpL[:], ones16[:], tf_sb[0:1, nf - 1:nf],
                                       tmpF[:], OP.mult, OP.subtract)
        nc.vector.tensor_scalar(dt_st[:], tmpL[:], 0.01, None, OP.mult)
        # force the exp/ln table load once, outside the interval loop
        nc.scalar.activation(tmpF[:], ones16[:], AF.Exp)
        nc.scalar.activation(tmpF[:], ones16[:], AF.Ln)

        def w0mm(dst, rhs32, start=True, stop=True):
            nc.tensor.matmul(dst, sb["w0T"][:, 0:128], rhs32[:, 0:BL], start=start, stop=False)
            nc.tensor.matmul(dst, sb["w0T"][:, 128:256], rhs32[:, BL:W2], start=False, stop=stop)

        def emit_eval(j, u_src):
            """u_src: AP for pre-activation of layer0 (psum or sbuf). Fills kk[j]."""
            nc.scalar.activation(e0[:], u_src, AF.Exp, bias=sb["b0c"][:, 0:1])
            nc.scalar.activation(h0[:], e0[:], AF.Ln, bias=1.0)
            nc.tensor.matmul(P3[:], sb["w1T"][:], h0[:], start=True, stop=True)
            nc.scalar.activation(e1[:], P3[:], AF.Exp, bias=sb["b1c"][:, 0:1])
            nc.scalar.activation(h1[:], e1[:], AF.Ln, bias=1.0)
            for half in (0, 1):
                o = P4[:, half * BL:(half + 1) * BL]
                nc.tensor.matmul(o, sb["w2T"][:, half * 128:(half + 1) * 128], h1[:],
                                 start=True, stop=False)
                nc.tensor.matmul(o, sb["b2r"][0:1, half * 128:(half + 1) * 128],
                                 ones16[:], start=False, stop=True)
            nc.scalar.activation(et[:], P4[:], AF.Exp, scale=-2.0)
            nc.gpsimd.tensor_scalar(dd[:], et[:], 0.5, 0.5, OP.mult, OP.add)
            # kk_j = dt * 2*sigmoid(2v) = dtb2 / (0.5 + 0.5*exp(-2v))
            nc.vector.reciprocal_approx_fast(out=rcd[:], in_=dd[:])
            nc.vector.tensor_tensor(kk[j][:], dtb2[:], rcd[:], OP.mult)
            if j <= 5:
                w0mm(P5[:], kk[j])
                for s2 in range(j + 1, 7):
                    nc.vector.scalar_tensor_tensor(
                        acc[s2][:], P5[:], float(A_TAB[s2][j - 1]), acc[s2][:],
                        OP.mult, OP.add)

        def emit_step(tnext_ap, sidx):
            # lane control at step start
            nc.vector.tensor_scalar(rem[:], t_st[:], -1.0, tnext_ap, OP.mult, OP.add)
            nc.vector.tensor_scalar(mx[:], rem[:], 0.0, None, OP.max)
            nc.vector.tensor_tensor(dt_use[:], dt_st[:], mx[:], OP.min)
            nc.vector.tensor_scalar(nd[:], rem[:], 1e-8, None, OP.is_gt)
            nc.vector.tensor_scalar(done[:], rem[:], 1e-8, None, OP.is_le)
            nc.vector.tensor_copy(dt2[0:1, 0:BL], dt_use[:])
            nc.vector.tensor_copy(dt2[0:1, BL:W2], dt_use[:])
            nc.tensor.matmul(P0[:], onesr[:], dt2[:], start=True, stop=True)
            nc.scalar.copy(dtb2[:], P0[:])
            nc.tensor.matmul(P1[:, 0:BL], sb["w0o"][:], dt_use[:], start=True, stop=True)
            w0mm(P2[:, 0:BL], z)
            nc.scalar.copy(ub[:], P2[:, 0:BL])
            for s in range(2, 7):
                nc.vector.scalar_tensor_tensor(acc[s][:], P1[:, 0:BL], -SUM_A[s],
                                               ub[:], OP.mult, OP.add)
            emit_eval(1, P2[:, 0:BL])
            for s in range(2, 7):
                emit_eval(s, acc[s][:])
                if s == 6:
                    break
            # y5 = I@z + sum B5_j kk_j + ones x (-SUM_B5 * dt)
            nc.tensor.matmul(P0[:], sb["sid"][:, 0:128], z[:], start=True, stop=False)
            for j, slot in SID_B5.items():
                nc.tensor.matmul(P0[:], sb["sid"][:, slot * 128:(slot + 1) * 128],
                                 kk[j][:], start=False, stop=False)
            nc.vector.tensor_scalar(sy[:], dt2[:], -SUM_B5, None, OP.mult)
            nc.tensor.matmul(P0[:], onesr[:], sy[:], start=False, stop=True)
            nc.scalar.copy(y5sb[:], P0[:])
            w0mm(P2[:, 0:BL], y5sb)
            emit_eval(7, P2[:, 0:BL])
            # err
            first = True
            for j, slot in SID_E.items():
                nc.tensor.matmul(P1[:], sb["sid"][:, slot * 128:(slot + 1) * 128],
                                 kk[j][:], start=first, stop=False)
                first = False
            nc.vector.tensor_scalar(se[:], dt2[:], -SUM_E, None, OP.mult)
            nc.tensor.matmul(P1[:], onesr[:], se[:], start=False, stop=True)
            # scale, msq
            nc.vector.tensor_tensor(mx1[:], z[:], y5sb[:], OP.max)
            nc.vector.tensor_tensor(mx2[:], z[:], y5sb[:], OP.min)
            nc.vector.scalar_tensor_tensor(scm[:], mx2[:], -1.0, mx1[:], OP.mult, OP.max)
            nc.gpsimd.tensor_scalar(scm[:], scm[:], RTOL, ATOL, OP.mult, OP.add)
            nc.vector.reciprocal_approx_fast(out=rsc[:], in_=scm[:])
            nc.vector.tensor_tensor(qt[:], P1[:], rsc[:], OP.mult)
            nc.vector.tensor_tensor(q2[:], qt[:], qt[:], OP.mult)
            nc.tensor.matmul(P2[0:1, 0:W2], onesc[:], q2[:], start=True, stop=True)
            nc.scalar.copy(msq32[:], P2[0:1, 0:W2])
            nc.vector.tensor_tensor(tm[:], msq32[0:1, 0:BL], msq32[0:1, BL:W2], OP.add)
            # factor = clip(0.9 * (msq)^-0.1, 0.2, 10); msq = tm/256
            nc.scalar.activation(lnm[:], tm[:], AF.Ln, scale=1.0 / 256.0, bias=eps24[0:1, 0:1])
            nc.scalar.activation(f0[:], lnm[:], AF.Exp, scale=-0.1)
            nc.vector.tensor_scalar(fac[:], f0[:], 0.9, 0.2, OP.mult, OP.max)
            nc.vector.tensor_scalar(fac2[:], fac[:], 10.0, None, OP.min)
            # accept = (msq <= 1) & notdone   (tm <= 256)
            nc.vector.tensor_scalar(le[:], tm[:], 256.0, None, OP.is_le)
            nc.vector.tensor_tensor(acc16[:], le[:], nd[:], OP.mult)
            nc.vector.tensor_copy(acc32[0:1, 0:BL], acc16[:])
            nc.vector.tensor_copy(acc32[0:1, BL:W2], acc16[:])
            nc.tensor.matmul(P0[:], onesr[:], acc32[:], start=True, stop=True)
            nc.vector.tensor_copy(accI[:], P0[:])
            nc.vector.copy_predicated(z[:], accI[:], y5sb[:])
            nc.vector.tensor_tensor(st_t[:], acc16[:], dt_use[:], OP.mult)
            nc.vector.tensor_tensor(t_st[:], t_st[:], st_t[:], OP.add)
            nc.vector.tensor_tensor(cand[:], dt_use[:], fac2[:], OP.mult)
            nc.vector.tensor_scalar(cand2[:], cand[:], 1e-6, None, OP.max)
            nc.vector.tensor_copy(doneI[:], done[:])
            nc.vector.copy_predicated(cand2[:], doneI[:], dt_st[:])
            nc.vector.tensor_copy(dt_st[:], cand2[:])
            # all-done flag for early exit
            nc.vector.tensor_scalar(remn[:], t_st[:], -1.0, tnext_ap, OP.mult, OP.add)
            nc.vector.tensor_scalar(ndn[:], remn[:], 1e-8, None, OP.is_gt)
            nc.vector.reduce_max(flag[:], ndn[:], axis=mybir.AxisListType.X)
            nc.vector.tensor_copy(flagi[sidx][:], flag[:])
            nc.vector.tensor_copy(flagi[sidx][:], flag[:])

        with tc.For_i(1, nf) as iv:
            tprev_ap = tf_sb[0:1, ds(iv - 1, 1)]
            tnext_ap = tf_sb[0:1, ds(iv, 1)]
            nc.vector.tensor_scalar(t_st[:], ones16[:], tprev_ap, None, OP.mult)
            emit_step(tnext_ap, 0)
            with ExitStack() as stk:
                for s in range(1, max_steps):
                    v = nc.values_load(flagi[s - 1][0:1, 0:1],
                                       skip_runtime_bounds_check=True)
                    stk.enter_context(tc.If(v > 0))
                    emit_step(tnext_ap, s)
            off = nc.snap(iv * W2)
            nc.vector.tensor_copy(zsave[:, ds(off, W2)], z[:])

    # ================= readout =================
    with tc.tile_pool(name="pr", bufs=2, space="PSUM") as pr, \
         tc.tile_pool(name="rs", bufs=2) as rs:
        for s in range(nf):
            rop = pr.tile([COUT, BL], F32, tag="rop")
            nc.tensor.matmul(rop[:], sb["roT"][:, 0:COUT], zsave[:, s * W2:s * W2 + BL],
                             start=True, stop=False)
            nc.tensor.matmul(rop[:], sb["roT"][:, COUT:2 * COUT],
                             zsave[:, s * W2 + BL:(s + 1) * W2], start=False, stop=False)
            nc.tensor.matmul(rop[:], sb["rob"][:], ones16[:], start=False, stop=True)
            rosb = rs.tile([COUT, BL], F32, tag="rosb")
            nc.scalar.copy(rosb[:], rop[:])
            rotp = pr.tile([BL, COUT], F32, tag="rotp")
            nc.tensor.transpose(rotp[:], rosb[:], sb["sid"][0:COUT, 0:COUT])
            nc.scalar.copy(ys_sb[:, s * COUT:(s + 1) * COUT], rotp[:])
    nc.sync.dma_start(out_d[:], ys_sb[:])

    ctx.close()
    return nc


_CACHE = {}


def _get_program():
    if "nc" not in _CACHE:
        nc = build_program()
        nc.compile()
        _CACHE["nc"] = nc
    return _CACHE["nc"]


def kernel(**inputs):
    nc = _get_program()
    w = _prep_weights(inputs)
    in_maps = []
    for c in range(NCORES):
        m = dict(w)
        m["xT"] = _prep_core_x(inputs["y_past"], c)
        in_maps.append(m)
    res = run_bass_kernel_spmd(nc, in_maps, list(range(NCORES)))
    out = np.stack([np.asarray(res.results[c]["out"]).reshape(BL, NF, COUT)
                    for c in range(NCORES)])
    return out.reshape(B, NF, COUT)



# revision 4
# speedup vs baseline: 2.0777x; 2.0777x over previous
"""GRU-ODE Trainium2 kernel: data-parallel over 8 NeuronCores (16 samples each).

v2: fp16 matmul operands everywhere (1 cycle/row, FWL weight loads, no fp32
double-pass), sigmoid-only GRU phase (tanh synthesized as 2*sigmoid(2x)-1 so a
single activation-table set covers the whole phase), FSAL Dormand-Prince (k7 of
an accepted step is reused as k1 of the next -> 6 MLP evals per RK step),
k-space stage accumulation, batched fp16 readout with host-side transpose.

Phases per core:
  1. GRU encoder: 512 sequential steps, hidden folded [128, 2*16].
  2. Adaptive DOPRI5 ODE solve: 32 intervals x up-to-16 RK steps with
     device-side early exit. Softplus = Ln(exp(u)+1) via the
     natural_log_exp table set; tanh head via Exp + reciprocal.
  3. Readout: two big fp16 matmuls over all 33 save points; host transposes.
"""
import sys
import numpy as np

sys.path.insert(0, "/root/.axon_site/_ro/trn_rl_repo")

import concourse.bass as bass
import concourse.bacc as bacc
import concourse.tile as tile
import concourse.mybir as mybir
from contextlib import ExitStack
from concourse.bass import ds
from concourse.bass_utils import run_bass_kernel_spmd

F32 = mybir.dt.float32
F16 = mybir.dt.float16
I32 = mybir.dt.int32
AF = mybir.ActivationFunctionType
OP = mybir.AluOpType

B, TIN, NF = 128, 512, 33
CIN, H, COUT, WIDTH = 64, 256, 64, 128
MAX_STEPS = 16
RTOL, ATOL = 1e-3, 1e-6
NCORES = 8
BL = B // NCORES  # 16 samples per core
W2 = 2 * BL       # 32: two hidden halves side by side

# Dormand-Prince 5(4) tableau
A_TAB = {
    2: [0.2],
    3: [3 / 40, 9 / 40],
    4: [44 / 45, -56 / 15, 32 / 9],
    5: [19372 / 6561, -25360 / 2187, 64448 / 6561, -212 / 729],
    6: [9017 / 3168, -355 / 33, 46732 / 5247, 49 / 176, -5103 / 18656],
}
B5_TAB = {1: 35 / 384, 3: 500 / 1113, 4: 125 / 192, 5: -2187 / 6784, 6: 11 / 84}
E_TAB = {1: 71 / 57600, 3: -71 / 16695, 4: 71 / 1920,
         5: -17253 / 339200, 6: 22 / 525, 7: -1 / 40}
SUM_A = {s: float(sum(A_TAB[s])) for s in A_TAB}
SUM_B5 = float(sum(B5_TAB.values()))
SUM_E = float(sum(E_TAB.values()))
# scaled-identity slots: 0 = I, 1..5 = B5 coeffs (j=1,3,4,5,6), 6..11 = E coeffs
SID_B5 = {j: i + 1 for i, j in enumerate([1, 3, 4, 5, 6])}
SID_E = {j: i + 6 for i, j in enumerate([1, 3, 4, 5, 6, 7])}
NSID = 12
RO_SPLIT = 272  # readout column split: 33*16 = 272 + 256 (psum bank limit)


def _prep_weights(inp):
    """Host-side: transform weights into the SBUF layouts the kernel wants."""
    h = lambda a: np.ascontiguousarray(a, dtype=np.float16)
    f = lambda a: np.ascontiguousarray(a, dtype=np.float32)
    wih, whh = np.asarray(inp["gru_wih"]), np.asarray(inp["gru_whh"])
    gb, bn = np.asarray(inp["gru_b"]), np.asarray(inp["gru_bn"])
    w0, b0 = np.asarray(inp["w0"]), np.asarray(inp["b0"])
    w1, b1 = np.asarray(inp["w1"]), np.asarray(inp["b1"])
    w2, b2 = np.asarray(inp["w2"]), np.asarray(inp["b2"])
    row, rob = np.asarray(inp["ro_w"]), np.asarray(inp["ro_b"])
    t = np.asarray(inp["t"])

    sid = np.zeros((128, NSID * 128), np.float32)
    eye = np.eye(128, dtype=np.float32)
    sid[:, 0:128] = eye
    for j, slot in SID_B5.items():
        sid[:, slot * 128:(slot + 1) * 128] = eye * np.float32(B5_TAB[j])
    for j, slot in SID_E.items():
        sid[:, slot * 128:(slot + 1) * 128] = eye * np.float32(E_TAB[j])

    w0T = w0.T  # [256, 128]
    roT = row.T  # [256, 64]
    return {
        "wihT": h(np.concatenate([wih.T, gb[None, :]], axis=0)),  # [65, 768]
        "whhT0": h(whh.T[:128]), "whhT1": h(whh.T[128:]),  # [128, 768]
        "bnr": h(bn[None, :]),  # [1, 256]
        "w0T": h(np.concatenate([w0T[:128], w0T[128:]], axis=1)),  # [128, 256]
        "w1T": h(w1.T),  # [128, 128]
        "w2T": h(w2.T),  # [128, 256]
        "b0c": f(b0[:, None]), "b1c": f(b1[:, None]),  # [128, 1]
        "b2r": h(b2[None, :]),  # [1, 256]
        "roT": h(np.concatenate([roT[:128], roT[128:]], axis=1)),  # [128, 128]
        "robr": h(rob[None, :]),  # [1, 64]
        "sid": h(sid),  # [128, NSID*128]
        "tf": f(t[TIN:][None, :]),  # [1, NF]
    }


def _prep_core_x(y_past, core):
    """y_past [B, TIN, CIN] -> xT_aug [65, TIN*16] fp16, col = t*16+b."""
    yc = np.asarray(y_past, np.float32)[core * BL:(core + 1) * BL]  # [16,T,64]
    xt = yc.transpose(2, 1, 0).reshape(CIN, -1)  # [64, T*16]
    return np.ascontiguousarray(np.concatenate(
        [xt, np.ones((1, xt.shape[1]), np.float32)], axis=0).astype(np.float16))


def build_program(tin=TIN, nf=NF, max_steps=MAX_STEPS):
    nc = bacc.Bacc("TRN2", target_bir_lowering=False, debug=False)
    d = {}
    d["xT"] = nc.dram_tensor("xT", [CIN + 1, tin * BL], F16, kind="ExternalInput")
    d["tf"] = nc.dram_tensor("tf", [1, nf], F32, kind="ExternalInput")
    for nm, shp, dt in [
            ("wihT", [65, 768], F16), ("whhT0", [128, 768], F16),
            ("whhT1", [128, 768], F16), ("bnr", [1, 256], F16),
            ("w0T", [128, 256], F16), ("w1T", [128, 128], F16),
            ("w2T", [128, 256], F16), ("b0c", [128, 1], F32),
            ("b1c", [128, 1], F32), ("b2r", [1, 256], F16),
            ("roT", [128, 128], F16), ("robr", [1, 64], F16),
            ("sid", [128, NSID * 128], F16)]:
        d[nm] = nc.dram_tensor(nm, shp, dt, kind="ExternalInput")
    out_d = nc.dram_tensor("out", [COUT, nf * BL], F32, kind="ExternalOutput")

    ctx = ExitStack()
    tc = ctx.enter_context(tile.TileContext(nc))
    wp = ctx.enter_context(tc.tile_pool(name="w", bufs=1))
    sp = ctx.enter_context(tc.tile_pool(name="s", bufs=1))

    # ---- load weights & inputs ----
    sb = {}
    for nm in ["wihT", "whhT0", "whhT1", "bnr", "w0T", "w1T", "w2T", "b0c",
               "b1c", "b2r", "roT", "robr", "sid", "tf"]:
        sb[nm] = wp.tile(list(d[nm].shape), d[nm].dtype, tag=nm, name=nm)
        nc.sync.dma_start(sb[nm][:], d[nm][:])
    xT = wp.tile([CIN + 1, tin * BL], F16, tag="xT")
    nchunk = 4
    cw = tin * BL // nchunk
    for k in range(nchunk):
        nc.sync.dma_start(xT[:, k * cw:(k + 1) * cw], d["xT"][:, k * cw:(k + 1) * cw])

    ones16 = wp.tile([1, BL], F32, tag="ones16")       # f32 lane constant
    ones16f = wp.tile([1, BL], F16, tag="ones16f")     # f16 bias-matmul rhs
    onesrf = wp.tile([1, 128], F16, tag="onesrf")      # f16 broadcast lhsT
    onescf = wp.tile([128, 1], F16, tag="onescf")      # f16 reduce lhsT
    onesw = wp.tile([1, RO_SPLIT], F16, tag="onesw")   # readout bias rhs
    eps24 = wp.tile([1, 1], F32, tag="eps24", name="eps24")
    nc.vector.memset(eps24[:], 1e-24)
    nc.vector.memset(ones16[:], 1.0)
    nc.vector.memset(ones16f[:], 1.0)
    nc.vector.memset(onesrf[:], 1.0)
    nc.vector.memset(onescf[:], 1.0)
    nc.vector.memset(onesw[:], 1.0)

    # ---- state tiles (fixed addresses; live across dynamic control flow) ----
    z = sp.tile([128, W2], F16, tag="z")          # folded [hidden-half | sample]
    fs = sp.tile([128, W2], F16, tag="fs")        # FSAL: 2*sigmoid(2*v(z)) = f(z)+1
    t_st = sp.tile([1, BL], F32, tag="t_st")
    dt_st = sp.tile([1, BL], F32, tag="dt_st")
    zsaveA = sp.tile([128, nf * BL], F16, tag="zsaveA")
    zsaveB = sp.tile([128, nf * BL], F16, tag="zsaveB")
    ys_sb = sp.tile([COUT, nf * BL], F32, tag="ys")

    MM = nc.tensor.matmul

    # ================= GRU phase =================
    with nc.named_scope("gru"), \
         tc.tile_pool(name="pg", bufs=1, space="PSUM") as pg, \
         tc.tile_pool(name="gs", bufs=1) as gs:
        GA = pg.tile([128, 4 * BL], F32, tag="GA")   # [ra | rb | ua | ub]
        PN = pg.tile([128, W2], F32, tag="PN")       # [hn_a | hn_b] (incl bn)
        PI = pg.tile([128, W2], F32, tag="PI")       # [inn_a | inn_b]
        rz = gs.tile([128, 4 * BL], F16, tag="rz")
        q3a = gs.tile([128, W2], F16, tag="q3a")
        q3c = gs.tile([128, W2], F16, tag="q3c")
        s2 = gs.tile([128, W2], F16, tag="s2")
        omz = gs.tile([128, W2], F16, tag="omz")
        zh = gs.tile([128, W2], F16, tag="zh")
        wsum = gs.tile([128, W2], F16, tag="wsum")
        sn = gs.tile([128, W2], F16, tag="sn")
        nc.vector.memset(z[:], 0.0)

        for t in range(tin):
            xs = xT[:, t * BL:(t + 1) * BL]
            za, zb = z[:, 0:BL], z[:, BL:W2]
            # inn (x-only, own tile): runs while step t-1's tail finishes.
            # PSUM accumulation groups within a tile must be sequential
            # (zero-region constraint), so each 16-col group completes
            # before the next one starts.
            MM(PI[:, 0:BL], sb["wihT"][:, 512:640], xs, start=True, stop=True)
            MM(PI[:, BL:W2], sb["wihT"][:, 640:768], xs, start=True, stop=True)
            # r gate first (its sigmoid gates the longest chain)
            MM(GA[:, 0:16], sb["wihT"][:, 0:128], xs, start=True, stop=False)
            MM(GA[:, 0:16], sb["whhT0"][:, 0:128], za, start=False, stop=False)
            MM(GA[:, 0:16], sb["whhT1"][:, 0:128], zb, start=False, stop=True)
            MM(GA[:, 16:32], sb["wihT"][:, 128:256], xs, start=True, stop=False)
            MM(GA[:, 16:32], sb["whhT0"][:, 128:256], za, start=False, stop=False)
            MM(GA[:, 16:32], sb["whhT1"][:, 128:256], zb, start=False, stop=True)
            # n-gate hidden part next (feeds q3 right after sigmoid(r));
            # bn folded in via per-partition bias rows
            MM(PN[:, 0:16], sb["bnr"][0:1, 0:128], ones16f[:], start=True, stop=False)
            MM(PN[:, 0:16], sb["whhT0"][:, 512:640], za, start=False, stop=False)
            MM(PN[:, 0:16], sb["whhT1"][:, 512:640], zb, start=False, stop=True)
            MM(PN[:, 16:32], sb["bnr"][0:1, 128:256], ones16f[:], start=True, stop=False)
            MM(PN[:, 16:32], sb["whhT0"][:, 640:768], za, start=False, stop=False)
            MM(PN[:, 16:32], sb["whhT1"][:, 640:768], zb, start=False, stop=True)
            # update gate last
            MM(GA[:, 32:48], sb["wihT"][:, 256:384], xs, start=True, stop=False)
            MM(GA[:, 32:48], sb["whhT0"][:, 256:384], za, start=False, stop=False)
            MM(GA[:, 32:48], sb["whhT1"][:, 256:384], zb, start=False, stop=True)
            MM(GA[:, 48:64], sb["wihT"][:, 384:512], xs, start=True, stop=False)
            MM(GA[:, 48:64], sb["whhT0"][:, 384:512], za, start=False, stop=False)
            MM(GA[:, 48:64], sb["whhT1"][:, 384:512], zb, start=False, stop=True)

            nc.scalar.activation(rz[:, 0:W2], GA[:, 0:W2], AF.Sigmoid)
            nc.scalar.activation(rz[:, W2:2 * W2], GA[:, W2:2 * W2], AF.Sigmoid)
            nc.vector.tensor_tensor(q3a[:], PN[:], rz[:, 0:W2], OP.mult)
            nc.vector.tensor_tensor(q3c[:], q3a[:], PI[:], OP.add)
            # n = tanh(q3) = 2*sigmoid(2*q3) - 1; z' = 2s*(1-u) + (u*z - (1-u))
            nc.scalar.activation(s2[:], q3c[:], AF.Sigmoid, scale=2.0)
            nc.gpsimd.tensor_scalar(omz[:], rz[:, W2:2 * W2], -1.0, 1.0, OP.mult, OP.add)
            nc.gpsimd.tensor_tensor(zh[:], rz[:, W2:2 * W2], z[:], OP.mult)
            nc.gpsimd.tensor_tensor(wsum[:], zh[:], omz[:], OP.subtract)
            nc.vector.scalar_tensor_tensor(sn[:], s2[:], 2.0, omz[:], OP.mult, OP.mult)
            nc.vector.tensor_tensor(z[:], sn[:], wsum[:], OP.add)

    nc.vector.tensor_copy(zsaveA[:, 0:BL], z[:, 0:BL])
    nc.vector.tensor_copy(zsaveB[:, 0:BL], z[:, BL:W2])

    # ================= ODE phase =================
    with nc.named_scope("ode"), \
         tc.tile_pool(name="po", bufs=1, space="PSUM") as po, \
         tc.tile_pool(name="osb", bufs=1) as osb:
        Pd = po.tile([128, W2], F32, tag="Pd")    # dt broadcast
        Pu = po.tile([128, BL], F32, tag="Pu")    # MLP pre-activations
        Pe = po.tile([128, BL], F32, tag="Pe")    # exp intermediates
        P4 = po.tile([128, W2], F32, tag="P4")    # head pre-activation
        P0 = po.tile([128, W2], F32, tag="P0")    # y5
        P1 = po.tile([128, W2], F32, tag="P1")    # err
        P2 = po.tile([1, W2], F32, tag="P2")      # msq partial
        Pa = po.tile([128, W2], F32, tag="Pa")    # accept broadcast

        dtb2 = osb.tile([128, W2], F16, tag="dtb2")
        dt2f = osb.tile([1, W2], F16, tag="dt2f")
        sy = osb.tile([1, W2], F16, tag="sy")
        se = osb.tile([1, W2], F16, tag="se")
        h0 = osb.tile([128, BL], F16, tag="h0")
        h1 = osb.tile([128, BL], F16, tag="h1")
        ed = osb.tile([128, W2], F32, tag="ed")
        dd = osb.tile([128, W2], F32, tag="dd")
        rcd = osb.tile([128, W2], F32, tag="rcd")
        kk = {j: osb.tile([128, W2], F16, tag=f"kk{j}", name=f"kk{j}")
              for j in range(1, 8)}
        zacc = {s: osb.tile([128, W2], F16, tag=f"zacc{s}", name=f"zacc{s}")
                for s in range(2, 7)}
        y5sb = osb.tile([128, W2], F16, tag="y5sb")
        fs_c = osb.tile([128, W2], F16, tag="fs_c")
        dz = osb.tile([128, W2], F16, tag="dz")
        zm = osb.tile([128, W2], F16, tag="zm")
        dfs = osb.tile([128, W2], F16, tag="dfs")
        fsm = osb.tile([128, W2], F16, tag="fsm")
        mx1 = osb.tile([128, W2], F32, tag="mx1")
        mx2 = osb.tile([128, W2], F32, tag="mx2")
        scm = osb.tile([128, W2], F32, tag="scm")
        rsc = osb.tile([128, W2], F32, tag="rsc")
        qt = osb.tile([128, W2], F32, tag="qt")
        q2 = osb.tile([128, W2], F16, tag="q2")
        msq32 = osb.tile([1, W2], F32, tag="msq32")
        acc32 = osb.tile([1, W2], F16, tag="acc32")
        L16 = lambda tg: osb.tile([1, BL], F32, tag=tg, name=tg)
        rem, mx, dt_use = L16("rem"), L16("mx"), L16("dt_use")
        nd, done = L16("nd"), L16("done")
        tm, lnm, f0 = L16("tm"), L16("lnm"), L16("f0")
        fac, fac2, le, acc16 = L16("fac"), L16("fac2"), L16("le"), L16("acc16")
        st_t, cand, cand2 = L16("st_t"), L16("cand"), L16("cand2")
        remn, ndn = L16("remn"), L16("ndn")
        doneI = osb.tile([1, BL], I32, tag="doneI")
        flag = osb.tile([1, 1], F32, tag="flag")
        flagi = [osb.tile([1, 1], I32, tag=f"flagi{s}", name=f"flagi{s}")
                 for s in range(max_steps)]
        tmpF, tmpL = L16("tmpF"), L16("tmpL")

        tf_sb = sb["tf"]
        # dt0 = (tf[-1] - tf[0]) * 0.01
        nc.vector.tensor_scalar(tmpF[:], ones16[:], tf_sb[0:1, 0:1], None, OP.mult)
        nc.vector.scalar_tensor_tensor(tmpL[:], ones16[:], tf_sb[0:1, nf - 1:nf],
                                       tmpF[:], OP.mult, OP.subtract)
        nc.vector.tensor_scalar(dt_st[:], tmpL[:], 0.01, None, OP.mult)
        # force the exp/ln table load once, outside the interval loop
        nc.scalar.activation(tmpF[:], ones16[:], AF.Exp)
        nc.scalar.activation(tmpF[:], ones16[:], AF.Ln)

        def emit_mlp(rhs, tail):
            """MLP eval on rhs [128, W2] fp16; tail(rcd) consumes 1/(0.5+0.5*e^-2v)."""
            MM(Pu[:], sb["w0T"][:, 0:128], rhs[:, 0:BL], start=True, stop=False)
            MM(Pu[:], sb["w0T"][:, 128:256], rhs[:, BL:W2], start=False, stop=True)
            nc.scalar.activation(Pe[:], Pu[:], AF.Exp, bias=sb["b0c"][:, 0:1])
            nc.scalar.activation(h0[:], Pe[:], AF.Ln, bias=1.0)
            MM(Pu[:], sb["w1T"][:], h0[:], start=True, stop=True)
            nc.scalar.activation(Pe[:], Pu[:], AF.Exp, bias=sb["b1c"][:, 0:1])
            nc.scalar.activation(h1[:], Pe[:], AF.Ln, bias=1.0)
            MM(P4[:, 0:BL], sb["w2T"][:, 0:128], h1[:], start=True, stop=False)
            MM(P4[:, 0:BL], sb["b2r"][0:1, 0:128], ones16f[:], start=False, stop=True)
            MM(P4[:, BL:W2], sb["w2T"][:, 128:256], h1[:], start=True, stop=False)
            MM(P4[:, BL:W2], sb["b2r"][0:1, 128:256], ones16f[:], start=False, stop=True)
            nc.scalar.activation(ed[:], P4[:], AF.Exp, scale=-2.0)
            nc.vector.tensor_scalar(dd[:], ed[:], 0.5, 0.5, OP.mult, OP.add)
            nc.vector.reciprocal_approx_fast(out=rcd[:], in_=dd[:])
            tail()

        def emit_step(tnext_ap, sidx):
            # lane control at step start (all f32 [1,16])
            nc.vector.tensor_scalar(rem[:], t_st[:], -1.0, tnext_ap, OP.mult, OP.add)
            nc.vector.tensor_scalar(mx[:], rem[:], 0.0, None, OP.max)
            nc.vector.tensor_tensor(dt_use[:], dt_st[:], mx[:], OP.min)
            nc.vector.tensor_scalar(nd[:], rem[:], 1e-8, None, OP.is_gt)
            nc.vector.tensor_scalar(done[:], rem[:], 1e-8, None, OP.is_le)
            nc.vector.tensor_copy(dt2f[0:1, 0:BL], dt_use[:])
            nc.vector.tensor_copy(dt2f[0:1, BL:W2], dt_use[:])
            MM(Pd[:], onesrf[:], dt2f[:], start=True, stop=True)
            nc.vector.tensor_copy(dtb2[:], Pd[:])
            nc.vector.tensor_scalar(sy[:], dt2f[:], -SUM_B5, None, OP.mult)
            nc.vector.tensor_scalar(se[:], dt2f[:], -SUM_E, None, OP.mult)
            # FSAL: kk1 = dt * (f(z)+1) = dt * fs, no MLP eval needed
            nc.vector.tensor_tensor(kk[1][:], dtb2[:], fs[:], OP.mult)
            MM(P1[:], sb["sid"][:, SID_E[1] * 128:(SID_E[1] + 1) * 128], kk[1][:],
               start=True, stop=False)
            MM(P0[:], sb["sid"][:, 0:128], z[:], start=True, stop=False)
            MM(P0[:], sb["sid"][:, SID_B5[1] * 128:(SID_B5[1] + 1) * 128], kk[1][:],
               start=False, stop=False)
            for s in range(2, 7):
                nc.vector.scalar_tensor_tensor(zacc[s][:], dtb2[:], -SUM_A[s],
                                               z[:], OP.mult, OP.add)
            for s in range(2, 7):
                nc.vector.scalar_tensor_tensor(zacc[s][:], kk[1][:], A_TAB[s][0],
                                               zacc[s][:], OP.mult, OP.add)

            for j in range(2, 7):
                def tail(j=j):
                    nc.vector.tensor_tensor(kk[j][:], dtb2[:], rcd[:], OP.mult)
                    for s2 in range(j + 1, 7):
                        nc.vector.scalar_tensor_tensor(
                            zacc[s2][:], kk[j][:], A_TAB[s2][j - 1], zacc[s2][:],
                            OP.mult, OP.add)
                    if j in SID_B5:
                        MM(P0[:], sb["sid"][:, SID_B5[j] * 128:(SID_B5[j] + 1) * 128],
                           kk[j][:], start=False, stop=False)
                    if j in SID_E:
                        MM(P1[:], sb["sid"][:, SID_E[j] * 128:(SID_E[j] + 1) * 128],
                           kk[j][:], start=False, stop=False)
                emit_mlp(zacc[j], tail)

            # y5 = I@z + sum B5_j kk_j - SUM_B5*dt
            MM(P0[:], onesrf[:], sy[:], start=False, stop=True)
            nc.vector.tensor_copy(y5sb[:], P0[:])
            # overlap with eval7: scale + dz
            nc.vector.tensor_tensor(mx1[:], z[:], y5sb[:], OP.max)
            nc.vector.tensor_tensor(mx2[:], z[:], y5sb[:], OP.min)
            nc.vector.scalar_tensor_tensor(scm[:], mx2[:], -1.0, mx1[:], OP.mult, OP.max)
            nc.vector.tensor_scalar(scm[:], scm[:], RTOL, ATOL, OP.mult, OP.add)
            nc.vector.reciprocal_approx_fast(out=rsc[:], in_=scm[:])
            nc.vector.tensor_tensor(dz[:], y5sb[:], z[:], OP.subtract)

            def tail7():
                nc.vector.tensor_tensor(kk[7][:], dtb2[:], rcd[:], OP.mult)
                nc.vector.tensor_copy(fs_c[:], rcd[:])
                MM(P1[:], onesrf[:], se[:], start=False, stop=False)
                MM(P1[:], sb["sid"][:, SID_E[7] * 128:(SID_E[7] + 1) * 128],
                   kk[7][:], start=False, stop=True)
            emit_mlp(y5sb, tail7)

            # error norm and controller
            nc.vector.tensor_tensor(qt[:], P1[:], rsc[:], OP.mult)
            nc.vector.tensor_tensor(q2[:], qt[:], qt[:], OP.mult)
            MM(P2[0:1, 0:W2], onescf[:], q2[:], start=True, stop=True)
            nc.vector.tensor_copy(msq32[:], P2[0:1, 0:W2])
            nc.vector.tensor_tensor(tm[:], msq32[0:1, 0:BL], msq32[0:1, BL:W2], OP.add)
            # factor = clip(0.9 * (tm/256)^-0.1, 0.2, 10)
            nc.scalar.activation(lnm[:], tm[:], AF.Ln, scale=1.0 / 256.0,
                                 bias=eps24[0:1, 0:1])
            nc.scalar.activation(f0[:], lnm[:], AF.Exp, scale=-0.1)
            nc.vector.tensor_scalar(fac[:], f0[:], 0.9, 0.2, OP.mult, OP.max)
            nc.vector.tensor_scalar(fac2[:], fac[:], 10.0, None, OP.min)
            # accept = (tm <= 256) & notdone
            nc.vector.tensor_scalar(le[:], tm[:], 256.0, None, OP.is_le)
            nc.vector.tensor_tensor(acc16[:], le[:], nd[:], OP.mult)
            nc.vector.tensor_copy(acc32[0:1, 0:BL], acc16[:])
            nc.vector.tensor_copy(acc32[0:1, BL:W2], acc16[:])
            MM(Pa[:], onesrf[:], acc32[:], start=True, stop=True)
            # masked state updates: x += accept * (cand - x)
            nc.vector.tensor_tensor(zm[:], Pa[:], dz[:], OP.mult)
            nc.vector.tensor_tensor(z[:], z[:], zm[:], OP.add)
            nc.vector.tensor_tensor(dfs[:], fs_c[:], fs[:], OP.subtract)
            nc.vector.tensor_tensor(fsm[:], Pa[:], dfs[:], OP.mult)
            nc.vector.tensor_tensor(fs[:], fs[:], fsm[:], OP.add)
            nc.vector.tensor_tensor(st_t[:], acc16[:], dt_use[:], OP.mult)
            nc.vector.tensor_tensor(t_st[:], t_st[:], st_t[:], OP.add)
            nc.vector.tensor_tensor(cand[:], dt_use[:], fac2[:], OP.mult)
            nc.vector.tensor_scalar(cand2[:], cand[:], 1e-6, None, OP.max)
            nc.vector.tensor_copy(doneI[:], done[:])
            nc.vector.copy_predicated(cand2[:], doneI[:], dt_st[:])
            nc.vector.tensor_copy(dt_st[:], cand2[:])
            # all-done flag for early exit
            nc.vector.tensor_scalar(remn[:], t_st[:], -1.0, tnext_ap, OP.mult, OP.add)
            nc.vector.tensor_scalar(ndn[:], remn[:], 1e-8, None, OP.is_gt)
            nc.vector.reduce_max(flag[:], ndn[:], axis=mybir.AxisListType.X)
            nc.vector.tensor_copy(flagi[sidx][:], flag[:])

        # initial FSAL eval: fs = 2*sigmoid(2*v(z))
        def tail0():
            nc.vector.tensor_copy(fs[:], rcd[:])
        emit_mlp(z, tail0)

        with tc.For_i(1, nf) as iv:
            tprev_ap = tf_sb[0:1, ds(iv - 1, 1)]
            tnext_ap = tf_sb[0:1, ds(iv, 1)]
            nc.vector.tensor_scalar(t_st[:], ones16[:], tprev_ap, None, OP.mult)
            emit_step(tnext_ap, 0)
            with ExitStack() as stk:
                for s in range(1, max_steps):
                    v = nc.values_load(flagi[s - 1][0:1, 0:1],
                                       skip_runtime_bounds_check=True)
                    stk.enter_context(tc.If(v > 0))
                    emit_step(tnext_ap, s)
            off = nc.snap(iv * BL)
            nc.vector.tensor_copy(zsaveA[:, ds(off, BL)], z[:, 0:BL])
            nc.vector.tensor_copy(zsaveB[:, ds(off, BL)], z[:, BL:W2])

    # ================= readout =================
    # ys[c, s*16+b] = (ro_w @ z_s)[c, b] + ro_b[c]; host transposes to [b, s, c]
    with nc.named_scope("readout"), \
         tc.tile_pool(name="pr", bufs=2, space="PSUM") as pr:
        for lo, hi in [(0, RO_SPLIT), (RO_SPLIT, nf * BL)]:
            w = hi - lo
            rop = pr.tile([COUT, RO_SPLIT], F32, tag="rop")
            MM(rop[:, 0:w], sb["roT"][:, 0:COUT], zsaveA[:, lo:hi],
               start=True, stop=False)
            MM(rop[:, 0:w], sb["roT"][:, COUT:2 * COUT], zsaveB[:, lo:hi],
               start=False, stop=False)
            MM(rop[:, 0:w], sb["robr"][:], onesw[0:1, 0:w], start=False, stop=True)
            nc.vector.tensor_copy(ys_sb[:, lo:hi], rop[:, 0:w])
    nc.sync.dma_start(out_d[:], ys_sb[:])

    ctx.close()
    return nc


_CACHE = {}


def _get_program():
    if "nc" not in _CACHE:
        nc = build_program()
        nc.compile()
        _CACHE["nc"] = nc
    return _CACHE["nc"]


def kernel(**inputs):
    nc = _get_program()
    w = _prep_weights(inputs)
    in_maps = []
    for c in range(NCORES):
        m = dict(w)
        m["xT"] = _prep_core_x(inputs["y_past"], c)
        in_maps.append(m)
    res = run_bass_kernel_spmd(nc, in_maps, list(range(NCORES)))
    out = np.stack([
        np.asarray(res.results[c]["out"]).reshape(COUT, NF, BL).transpose(2, 1, 0)
        for c in range(NCORES)])
    return out.reshape(B, NF, COUT).astype(np.float32)


# revision 6
# speedup vs baseline: 2.8655x; 1.3792x over previous
"""GRU-ODE Trainium2 kernel: data-parallel over 8 NeuronCores (16 samples each).

v2: fp16 matmul operands everywhere (1 cycle/row, FWL weight loads, no fp32
double-pass), sigmoid-only GRU phase (tanh synthesized as 2*sigmoid(2x)-1 so a
single activation-table set covers the whole phase), FSAL Dormand-Prince (k7 of
an accepted step is reused as k1 of the next -> 6 MLP evals per RK step),
k-space stage accumulation, batched fp16 readout with host-side transpose.

Phases per core:
  1. GRU encoder: 512 sequential steps, hidden folded [128, 2*16].
  2. Adaptive DOPRI5 ODE solve: 32 intervals x up-to-16 RK steps with
     device-side early exit. Softplus = Ln(exp(u)+1) via the
     natural_log_exp table set; tanh head via Exp + reciprocal.
  3. Readout: two big fp16 matmuls over all 33 save points; host transposes.
"""
import sys
import numpy as np

sys.path.insert(0, "/root/.axon_site/_ro/trn_rl_repo")

import concourse.bass as bass
import concourse.bacc as bacc
import concourse.tile as tile
import concourse.mybir as mybir
from contextlib import ExitStack
from concourse.bass import ds
from concourse.bass_utils import run_bass_kernel_spmd

F32 = mybir.dt.float32
F16 = mybir.dt.float16
I32 = mybir.dt.int32
AF = mybir.ActivationFunctionType
OP = mybir.AluOpType

B, TIN, NF = 128, 512, 33
CIN, H, COUT, WIDTH = 64, 256, 64, 128
MAX_STEPS = 16
RTOL, ATOL = 1e-3, 1e-6
NCORES = 8
BL = B // NCORES  # 16 samples per core
W2 = 2 * BL       # 32: two hidden halves side by side

# Dormand-Prince 5(4) tableau
A_TAB = {
    2: [0.2],
    3: [3 / 40, 9 / 40],
    4: [44 / 45, -56 / 15, 32 / 9],
    5: [19372 / 6561, -25360 / 2187, 64448 / 6561, -212 / 729],
    6: [9017 / 3168, -355 / 33, 46732 / 5247, 49 / 176, -5103 / 18656],
}
B5_TAB = {1: 35 / 384, 3: 500 / 1113, 4: 125 / 192, 5: -2187 / 6784, 6: 11 / 84}
E_TAB = {1: 71 / 57600, 3: -71 / 16695, 4: 71 / 1920,
         5: -17253 / 339200, 6: 22 / 525, 7: -1 / 40}
SUM_A = {s: float(sum(A_TAB[s])) for s in A_TAB}
SUM_B5 = float(sum(B5_TAB.values()))
SUM_E = float(sum(E_TAB.values()))
# scaled-identity slots: 0 = I, 1..5 = B5 coeffs (j=1,3,4,5,6), 6..11 = E coeffs
SID_B5 = {j: i + 1 for i, j in enumerate([1, 3, 4, 5, 6])}
SID_E = {j: i + 6 for i, j in enumerate([1, 3, 4, 5, 6, 7])}
NSID = 12
RO_SPLIT = 272  # readout column split: 33*16 = 272 + 256 (psum bank limit)


def _prep_weights(inp):
    """Host-side: transform weights into the SBUF layouts the kernel wants."""
    h = lambda a: np.ascontiguousarray(a, dtype=np.float16)
    f = lambda a: np.ascontiguousarray(a, dtype=np.float32)
    wih, whh = np.asarray(inp["gru_wih"]), np.asarray(inp["gru_whh"])
    gb, bn = np.asarray(inp["gru_b"]), np.asarray(inp["gru_bn"])
    w0, b0 = np.asarray(inp["w0"]), np.asarray(inp["b0"])
    w1, b1 = np.asarray(inp["w1"]), np.asarray(inp["b1"])
    w2, b2 = np.asarray(inp["w2"]), np.asarray(inp["b2"])
    row, rob = np.asarray(inp["ro_w"]), np.asarray(inp["ro_b"])
    t = np.asarray(inp["t"])

    sid = np.zeros((128, NSID * 128), np.float32)
    eye = np.eye(128, dtype=np.float32)
    sid[:, 0:128] = eye
    for j, slot in SID_B5.items():
        sid[:, slot * 128:(slot + 1) * 128] = eye * np.float32(B5_TAB[j])
    for j, slot in SID_E.items():
        sid[:, slot * 128:(slot + 1) * 128] = eye * np.float32(E_TAB[j])

    w0T = w0.T  # [256, 128]
    roT = row.T  # [256, 64]
    return {
        "wihT": h(np.concatenate([wih.T, gb[None, :]], axis=0)),  # [65, 768]
        "whhT0": h(whh.T[:128]), "whhT1": h(whh.T[128:]),  # [128, 768]
        "bnr": h(bn[None, :]),  # [1, 256]
        "w0T": h(np.concatenate([w0T[:128], w0T[128:]], axis=1)),  # [128, 256]
        "w1T": h(w1.T),  # [128, 128]
        "w2T": h(w2.T),  # [128, 256]
        "b0c": f(b0[:, None]), "b1c": f(b1[:, None]),  # [128, 1]
        "b2r": h(b2[None, :]),  # [1, 256]
        "roT": h(np.concatenate([roT[:128], roT[128:]], axis=1)),  # [128, 128]
        "robr": h(rob[None, :]),  # [1, 64]
        "sid": h(sid),  # [128, NSID*128]
        "tf": f(t[TIN:][None, :]),  # [1, NF]
    }


def _prep_core_x(y_past, core):
    """y_past [B, TIN, CIN] -> xT_aug [65, TIN*16] fp16, col = t*16+b."""
    yc = np.asarray(y_past, np.float32)[core * BL:(core + 1) * BL]  # [16,T,64]
    xt = yc.transpose(2, 1, 0).reshape(CIN, -1)  # [64, T*16]
    return np.ascontiguousarray(np.concatenate(
        [xt, np.ones((1, xt.shape[1]), np.float32)], axis=0).astype(np.float16))


def _pin_exp_ln_tables(arch):
    """Make natural_log_exp_and_others the only table set advertising Exp/Ln.

    The act-table-load pass keeps the current set when it suffices, else picks
    the FIRST set containing the function. Exp's first match (exp_and_others)
    lacks Ln and vice versa, so Exp<->Ln chains thrash ACT_TABLE_LOAD (~1.3us
    each). Removing exp/ln from the other sets' membership (contents only --
    set order and ids unchanged) forces the one set that truly has both.
    """
    from concourse.hw_specs import get_activation_tables
    tabs = get_activation_tables(arch)  # functools.cache: mutate in place
    for name, fns in tabs.items():
        if name == "natural_log_exp_and_others":
            continue
        fns.discard(AF.Exp)
        fns.discard(AF.Ln)


def build_program(tin=TIN, nf=NF, max_steps=MAX_STEPS):
    nc = bacc.Bacc("TRN2", target_bir_lowering=False, debug=False)
    _pin_exp_ln_tables(nc.m.arch)
    d = {}
    d["xT"] = nc.dram_tensor("xT", [CIN + 1, tin * BL], F16, kind="ExternalInput")
    d["tf"] = nc.dram_tensor("tf", [1, nf], F32, kind="ExternalInput")
    for nm, shp, dt in [
            ("wihT", [65, 768], F16), ("whhT0", [128, 768], F16),
            ("whhT1", [128, 768], F16), ("bnr", [1, 256], F16),
            ("w0T", [128, 256], F16), ("w1T", [128, 128], F16),
            ("w2T", [128, 256], F16), ("b0c", [128, 1], F32),
            ("b1c", [128, 1], F32), ("b2r", [1, 256], F16),
            ("roT", [128, 128], F16), ("robr", [1, 64], F16),
            ("sid", [128, NSID * 128], F16)]:
        d[nm] = nc.dram_tensor(nm, shp, dt, kind="ExternalInput")
    out_d = nc.dram_tensor("out", [COUT, nf * BL], F32, kind="ExternalOutput")

    ctx = ExitStack()
    tc = ctx.enter_context(tile.TileContext(nc))
    wp = ctx.enter_context(tc.tile_pool(name="w", bufs=1))
    sp = ctx.enter_context(tc.tile_pool(name="s", bufs=1))

    # ---- load weights & inputs ----
    sb = {}
    for nm in ["wihT", "whhT0", "whhT1", "bnr", "w0T", "w1T", "w2T", "b0c",
               "b1c", "b2r", "roT", "robr", "sid", "tf"]:
        sb[nm] = wp.tile(list(d[nm].shape), d[nm].dtype, tag=nm, name=nm)
        nc.sync.dma_start(sb[nm][:], d[nm][:])
    xT = wp.tile([CIN + 1, tin * BL], F16, tag="xT")
    nchunk = 4
    cw = tin * BL // nchunk
    for k in range(nchunk):
        nc.sync.dma_start(xT[:, k * cw:(k + 1) * cw], d["xT"][:, k * cw:(k + 1) * cw])

    ones16 = wp.tile([1, BL], F32, tag="ones16")       # f32 lane constant
    ones16f = wp.tile([1, BL], F16, tag="ones16f")     # f16 bias-matmul rhs
    onesrf = wp.tile([1, 128], F16, tag="onesrf")      # f16 broadcast lhsT
    onescf = wp.tile([128, 1], F16, tag="onescf")      # f16 reduce lhsT
    onesw = wp.tile([1, RO_SPLIT], F16, tag="onesw")   # readout bias rhs
    eps24 = wp.tile([1, 1], F32, tag="eps24", name="eps24")
    nc.vector.memset(eps24[:], 1e-24)
    nc.vector.memset(ones16[:], 1.0)
    nc.vector.memset(ones16f[:], 1.0)
    nc.vector.memset(onesrf[:], 1.0)
    nc.vector.memset(onescf[:], 1.0)
    nc.vector.memset(onesw[:], 1.0)

    # ---- state tiles (fixed addresses; live across dynamic control flow) ----
    z = sp.tile([128, W2], F16, tag="z")          # folded [hidden-half | sample]
    fs = sp.tile([128, W2], F16, tag="fs")        # FSAL: 2*sigmoid(2*v(z)) = f(z)+1
    t_st = sp.tile([1, BL], F32, tag="t_st")
    dt_st = sp.tile([1, BL], F32, tag="dt_st")
    zsaveA = sp.tile([128, nf * BL], F16, tag="zsaveA")
    zsaveB = sp.tile([128, nf * BL], F16, tag="zsaveB")
    ys_sb = sp.tile([COUT, nf * BL], F32, tag="ys")

    MM = nc.tensor.matmul

    # ================= GRU phase =================
    with nc.named_scope("gru"), \
         tc.tile_pool(name="pg", bufs=1, space="PSUM") as pg, \
         tc.tile_pool(name="gs", bufs=1) as gs:
        GA = pg.tile([128, 4 * BL], F32, tag="GA")   # [ra | rb | ua | ub]
        PN = pg.tile([128, W2], F32, tag="PN")       # [hn_a | hn_b] (incl bn)
        PI = pg.tile([128, W2], F32, tag="PI")       # [inn_a | inn_b]
        rz = gs.tile([128, 4 * BL], F16, tag="rz")
        q3a = gs.tile([128, W2], F16, tag="q3a")
        q3c = gs.tile([128, W2], F16, tag="q3c")
        s2 = gs.tile([128, W2], F16, tag="s2")
        omz = gs.tile([128, W2], F16, tag="omz")
        zh = gs.tile([128, W2], F16, tag="zh")
        wsum = gs.tile([128, W2], F16, tag="wsum")
        sn = gs.tile([128, W2], F16, tag="sn")
        nc.vector.memset(z[:], 0.0)

        for t in range(tin):
            xs = xT[:, t * BL:(t + 1) * BL]
            za, zb = z[:, 0:BL], z[:, BL:W2]
            # inn (x-only, own tile): runs while step t-1's tail finishes.
            # PSUM accumulation groups within a tile must be sequential
            # (zero-region constraint), so each 16-col group completes
            # before the next one starts.
            MM(PI[:, 0:BL], sb["wihT"][:, 512:640], xs, start=True, stop=True)
            MM(PI[:, BL:W2], sb["wihT"][:, 640:768], xs, start=True, stop=True)
            # r gate first (its sigmoid gates the longest chain)
            MM(GA[:, 0:16], sb["wihT"][:, 0:128], xs, start=True, stop=False)
            MM(GA[:, 0:16], sb["whhT0"][:, 0:128], za, start=False, stop=False)
            MM(GA[:, 0:16], sb["whhT1"][:, 0:128], zb, start=False, stop=True)
            MM(GA[:, 16:32], sb["wihT"][:, 128:256], xs, start=True, stop=False)
            MM(GA[:, 16:32], sb["whhT0"][:, 128:256], za, start=False, stop=False)
            MM(GA[:, 16:32], sb["whhT1"][:, 128:256], zb, start=False, stop=True)
            # n-gate hidden part next (feeds q3 right after sigmoid(r));
            # bn folded in via per-partition bias rows
            MM(PN[:, 0:16], sb["bnr"][0:1, 0:128], ones16f[:], start=True, stop=False)
            MM(PN[:, 0:16], sb["whhT0"][:, 512:640], za, start=False, stop=False)
            MM(PN[:, 0:16], sb["whhT1"][:, 512:640], zb, start=False, stop=True)
            MM(PN[:, 16:32], sb["bnr"][0:1, 128:256], ones16f[:], start=True, stop=False)
            MM(PN[:, 16:32], sb["whhT0"][:, 640:768], za, start=False, stop=False)
            MM(PN[:, 16:32], sb["whhT1"][:, 640:768], zb, start=False, stop=True)
            # update gate last
            MM(GA[:, 32:48], sb["wihT"][:, 256:384], xs, start=True, stop=False)
            MM(GA[:, 32:48], sb["whhT0"][:, 256:384], za, start=False, stop=False)
            MM(GA[:, 32:48], sb["whhT1"][:, 256:384], zb, start=False, stop=True)
            MM(GA[:, 48:64], sb["wihT"][:, 384:512], xs, start=True, stop=False)
            MM(GA[:, 48:64], sb["whhT0"][:, 384:512], za, start=False, stop=False)
            MM(GA[:, 48:64], sb["whhT1"][:, 384:512], zb, start=False, stop=True)

            nc.scalar.activation(rz[:, 0:W2], GA[:, 0:W2], AF.Sigmoid)
            nc.scalar.activation(rz[:, W2:2 * W2], GA[:, W2:2 * W2], AF.Sigmoid)
            nc.vector.tensor_tensor(q3a[:], PN[:], rz[:, 0:W2], OP.mult)
            nc.vector.tensor_tensor(q3c[:], q3a[:], PI[:], OP.add)
            # n = tanh(q3) = 2*sigmoid(2*q3) - 1; z' = 2s*(1-u) + (u*z - (1-u))
            nc.scalar.activation(s2[:], q3c[:], AF.Sigmoid, scale=2.0)
            nc.vector.tensor_scalar(omz[:], rz[:, W2:2 * W2], -1.0, 1.0, OP.mult, OP.add)
            nc.vector.tensor_tensor(zh[:], rz[:, W2:2 * W2], z[:], OP.mult)
            nc.vector.tensor_tensor(wsum[:], zh[:], omz[:], OP.subtract)
            nc.vector.scalar_tensor_tensor(sn[:], s2[:], 2.0, omz[:], OP.mult, OP.mult)
            nc.vector.tensor_tensor(z[:], sn[:], wsum[:], OP.add)

    nc.vector.tensor_copy(zsaveA[:, 0:BL], z[:, 0:BL])
    nc.vector.tensor_copy(zsaveB[:, 0:BL], z[:, BL:W2])

    # ================= ODE phase =================
    with nc.named_scope("ode"), \
         tc.tile_pool(name="po", bufs=1, space="PSUM") as po, \
         tc.tile_pool(name="osb", bufs=1) as osb:
        Pd = po.tile([128, W2], F32, tag="Pd")    # dt broadcast
        Pu = po.tile([128, BL], F32, tag="Pu")    # MLP pre-activations
        Pe = po.tile([128, BL], F32, tag="Pe")    # exp intermediates
        P4 = po.tile([128, W2], F32, tag="P4")    # head pre-activation
        P0 = po.tile([128, W2], F32, tag="P0")    # y5
        P1 = po.tile([128, W2], F32, tag="P1")    # err
        P2 = po.tile([1, W2], F32, tag="P2")      # msq partial
        Pa = po.tile([128, W2], F32, tag="Pa")    # accept broadcast

        dtb2 = osb.tile([128, W2], F16, tag="dtb2")
        dt2f = osb.tile([1, W2], F16, tag="dt2f")
        sy = osb.tile([1, W2], F16, tag="sy")
        se = osb.tile([1, W2], F16, tag="se")
        h0 = osb.tile([128, BL], F16, tag="h0")
        h1 = osb.tile([128, BL], F16, tag="h1")
        ed = osb.tile([128, W2], F32, tag="ed")
        dd = osb.tile([128, W2], F32, tag="dd")
        rcd = osb.tile([128, W2], F32, tag="rcd")
        kk = {j: osb.tile([128, W2], F16, tag=f"kk{j}", name=f"kk{j}")
              for j in range(1, 8)}
        zacc = {s: osb.tile([128, W2], F16, tag=f"zacc{s}", name=f"zacc{s}")
                for s in range(2, 7)}
        y5sb = osb.tile([128, W2], F16, tag="y5sb")
        fs_c = osb.tile([128, W2], F16, tag="fs_c")
        dz = osb.tile([128, W2], F16, tag="dz")
        zm = osb.tile([128, W2], F16, tag="zm")
        dfs = osb.tile([128, W2], F16, tag="dfs")
        fsm = osb.tile([128, W2], F16, tag="fsm")
        mx1 = osb.tile([128, W2], F32, tag="mx1")
        mx2 = osb.tile([128, W2], F32, tag="mx2")
        scm = osb.tile([128, W2], F32, tag="scm")
        rsc = osb.tile([128, W2], F32, tag="rsc")
        qt = osb.tile([128, W2], F32, tag="qt")
        q2 = osb.tile([128, W2], F16, tag="q2")
        msq32 = osb.tile([1, W2], F32, tag="msq32")
        acc32 = osb.tile([1, W2], F16, tag="acc32")
        L16 = lambda tg: osb.tile([1, BL], F32, tag=tg, name=tg)
        rem, mx, dt_use = L16("rem"), L16("mx"), L16("dt_use")
        nd, done = L16("nd"), L16("done")
        tm, lnm, f0 = L16("tm"), L16("lnm"), L16("f0")
        fac, fac2, le, acc16 = L16("fac"), L16("fac2"), L16("le"), L16("acc16")
        st_t, cand, cand2 = L16("st_t"), L16("cand"), L16("cand2")
        remn, ndn = L16("remn"), L16("ndn")
        doneI = osb.tile([1, BL], I32, tag="doneI")
        flag = osb.tile([1, 1], F32, tag="flag")
        flagi = [osb.tile([1, 1], I32, tag=f"flagi{s}", name=f"flagi{s}")
                 for s in range(max_steps)]
        tmpF, tmpL = L16("tmpF"), L16("tmpL")

        tf_sb = sb["tf"]
        # dt0 = (tf[-1] - tf[0]) * 0.01
        nc.vector.tensor_scalar(tmpF[:], ones16[:], tf_sb[0:1, 0:1], None, OP.mult)
        nc.vector.scalar_tensor_tensor(tmpL[:], ones16[:], tf_sb[0:1, nf - 1:nf],
                                       tmpF[:], OP.mult, OP.subtract)
        nc.vector.tensor_scalar(dt_st[:], tmpL[:], 0.01, None, OP.mult)
        # force the exp/ln table load once, outside the interval loop
        nc.scalar.activation(tmpF[:], ones16[:], AF.Exp)
        nc.scalar.activation(tmpF[:], ones16[:], AF.Ln)

        def emit_mlp(rhs, tail):
            """MLP eval on rhs [128, W2] fp16; tail(rcd) consumes 1/(0.5+0.5*e^-2v)."""
            MM(Pu[:], sb["w0T"][:, 0:128], rhs[:, 0:BL], start=True, stop=False)
            MM(Pu[:], sb["w0T"][:, 128:256], rhs[:, BL:W2], start=False, stop=True)
            nc.scalar.activation(Pe[:], Pu[:], AF.Exp, bias=sb["b0c"][:, 0:1])
            nc.scalar.activation(h0[:], Pe[:], AF.Ln, bias=1.0)
            MM(Pu[:], sb["w1T"][:], h0[:], start=True, stop=True)
            nc.scalar.activation(Pe[:], Pu[:], AF.Exp, bias=sb["b1c"][:, 0:1])
            nc.scalar.activation(h1[:], Pe[:], AF.Ln, bias=1.0)
            MM(P4[:, 0:BL], sb["w2T"][:, 0:128], h1[:], start=True, stop=False)
            MM(P4[:, 0:BL], sb["b2r"][0:1, 0:128], ones16f[:], start=False, stop=True)
            MM(P4[:, BL:W2], sb["w2T"][:, 128:256], h1[:], start=True, stop=False)
            MM(P4[:, BL:W2], sb["b2r"][0:1, 128:256], ones16f[:], start=False, stop=True)
            nc.scalar.activation(ed[:], P4[:], AF.Exp, scale=-2.0)
            nc.vector.tensor_scalar(dd[:], ed[:], 0.5, 0.5, OP.mult, OP.add)
            nc.vector.reciprocal_approx_fast(out=rcd[:], in_=dd[:])
            tail()

        def emit_step(tnext_ap, sidx):
            # lane control at step start (all f32 [1,16])
            nc.vector.tensor_scalar(rem[:], t_st[:], -1.0, tnext_ap, OP.mult, OP.add)
            nc.vector.tensor_scalar(mx[:], rem[:], 0.0, None, OP.max)
            nc.vector.tensor_tensor(dt_use[:], dt_st[:], mx[:], OP.min)
            nc.vector.tensor_scalar(nd[:], rem[:], 1e-8, None, OP.is_gt)
            nc.vector.tensor_scalar(done[:], rem[:], 1e-8, None, OP.is_le)
            nc.vector.tensor_copy(dt2f[0:1, 0:BL], dt_use[:])
            nc.vector.tensor_copy(dt2f[0:1, BL:W2], dt_use[:])
            MM(Pd[:], onesrf[:], dt2f[:], start=True, stop=True)
            nc.vector.tensor_copy(dtb2[:], Pd[:])
            nc.vector.tensor_scalar(sy[:], dt2f[:], -SUM_B5, None, OP.mult)
            nc.vector.tensor_scalar(se[:], dt2f[:], -SUM_E, None, OP.mult)
            # FSAL: kk1 = dt * (f(z)+1) = dt * fs, no MLP eval needed
            nc.vector.tensor_tensor(kk[1][:], dtb2[:], fs[:], OP.mult)
            MM(P1[:], sb["sid"][:, SID_E[1] * 128:(SID_E[1] + 1) * 128], kk[1][:],
               start=True, stop=False)
            MM(P0[:], sb["sid"][:, 0:128], z[:], start=True, stop=False)
            MM(P0[:], sb["sid"][:, SID_B5[1] * 128:(SID_B5[1] + 1) * 128], kk[1][:],
               start=False, stop=False)
            for s in range(2, 7):
                nc.vector.scalar_tensor_tensor(zacc[s][:], dtb2[:], -SUM_A[s],
                                               z[:], OP.mult, OP.add)
            for s in range(2, 7):
                nc.vector.scalar_tensor_tensor(zacc[s][:], kk[1][:], A_TAB[s][0],
                                               zacc[s][:], OP.mult, OP.add)

            for j in range(2, 7):
                def tail(j=j):
                    nc.vector.tensor_tensor(kk[j][:], dtb2[:], rcd[:], OP.mult)
                    for s2 in range(j + 1, 7):
                        nc.vector.scalar_tensor_tensor(
                            zacc[s2][:], kk[j][:], A_TAB[s2][j - 1], zacc[s2][:],
                            OP.mult, OP.add)
                    if j in SID_B5:
                        MM(P0[:], sb["sid"][:, SID_B5[j] * 128:(SID_B5[j] + 1) * 128],
                           kk[j][:], start=False, stop=False)
                    if j in SID_E:
                        MM(P1[:], sb["sid"][:, SID_E[j] * 128:(SID_E[j] + 1) * 128],
                           kk[j][:], start=False, stop=False)
                emit_mlp(zacc[j], tail)

            # y5 = I@z + sum B5_j kk_j - SUM_B5*dt
            MM(P0[:], onesrf[:], sy[:], start=False, stop=True)
            nc.vector.tensor_copy(y5sb[:], P0[:])
            # overlap with eval7: scale + dz
            nc.vector.tensor_tensor(mx1[:], z[:], y5sb[:], OP.max)
            nc.vector.tensor_tensor(mx2[:], z[:], y5sb[:], OP.min)
            nc.vector.scalar_tensor_tensor(scm[:], mx2[:], -1.0, mx1[:], OP.mult, OP.max)
            nc.vector.tensor_scalar(scm[:], scm[:], RTOL, ATOL, OP.mult, OP.add)
            nc.vector.reciprocal_approx_fast(out=rsc[:], in_=scm[:])
            nc.vector.tensor_tensor(dz[:], y5sb[:], z[:], OP.subtract)

            def tail7():
                nc.vector.tensor_tensor(kk[7][:], dtb2[:], rcd[:], OP.mult)
                nc.vector.tensor_copy(fs_c[:], rcd[:])
                MM(P1[:], onesrf[:], se[:], start=False, stop=False)
                MM(P1[:], sb["sid"][:, SID_E[7] * 128:(SID_E[7] + 1) * 128],
                   kk[7][:], start=False, stop=True)
            emit_mlp(y5sb, tail7)

            # error norm and controller
            nc.vector.tensor_tensor(qt[:], P1[:], rsc[:], OP.mult)
            nc.vector.tensor_tensor(q2[:], qt[:], qt[:], OP.mult)
            MM(P2[0:1, 0:W2], onescf[:], q2[:], start=True, stop=True)
            nc.vector.tensor_copy(msq32[:], P2[0:1, 0:W2])
            nc.vector.tensor_tensor(tm[:], msq32[0:1, 0:BL], msq32[0:1, BL:W2], OP.add)
            # factor = clip(0.9 * (tm/256)^-0.1, 0.2, 10)
            nc.scalar.activation(lnm[:], tm[:], AF.Ln, scale=1.0 / 256.0,
                                 bias=eps24[0:1, 0:1])
            nc.scalar.activation(f0[:], lnm[:], AF.Exp, scale=-0.1)
            nc.vector.tensor_scalar(fac[:], f0[:], 0.9, 0.2, OP.mult, OP.max)
            nc.vector.tensor_scalar(fac2[:], fac[:], 10.0, None, OP.min)
            # accept = (tm <= 256) & notdone
            nc.vector.tensor_scalar(le[:], tm[:], 256.0, None, OP.is_le)
            nc.vector.tensor_tensor(acc16[:], le[:], nd[:], OP.mult)
            nc.vector.tensor_copy(acc32[0:1, 0:BL], acc16[:])
            nc.vector.tensor_copy(acc32[0:1, BL:W2], acc16[:])
            MM(Pa[:], onesrf[:], acc32[:], start=True, stop=True)
            # masked state updates: x += accept * (cand - x)
            nc.vector.tensor_tensor(zm[:], Pa[:], dz[:], OP.mult)
            nc.vector.tensor_tensor(z[:], z[:], zm[:], OP.add)
            nc.vector.tensor_tensor(dfs[:], fs_c[:], fs[:], OP.subtract)
            nc.vector.tensor_tensor(fsm[:], Pa[:], dfs[:], OP.mult)
            nc.vector.tensor_tensor(fs[:], fs[:], fsm[:], OP.add)
            nc.vector.tensor_tensor(st_t[:], acc16[:], dt_use[:], OP.mult)
            nc.vector.tensor_tensor(t_st[:], t_st[:], st_t[:], OP.add)
            nc.vector.tensor_tensor(cand[:], dt_use[:], fac2[:], OP.mult)
            nc.vector.tensor_scalar(cand2[:], cand[:], 1e-6, None, OP.max)
            nc.vector.tensor_copy(doneI[:], done[:])
            nc.vector.copy_predicated(cand2[:], doneI[:], dt_st[:])
            nc.vector.tensor_copy(dt_st[:], cand2[:])
            # all-done flag for early exit
            nc.vector.tensor_scalar(remn[:], t_st[:], -1.0, tnext_ap, OP.mult, OP.add)
            nc.vector.tensor_scalar(ndn[:], remn[:], 1e-8, None, OP.is_gt)
            nc.vector.reduce_max(flag[:], ndn[:], axis=mybir.AxisListType.X)
            nc.vector.tensor_copy(flagi[sidx][:], flag[:])

        # initial FSAL eval: fs = 2*sigmoid(2*v(z))
        def tail0():
            nc.vector.tensor_copy(fs[:], rcd[:])
        emit_mlp(z, tail0)

        with tc.For_i(1, nf) as iv:
            tprev_ap = tf_sb[0:1, ds(iv - 1, 1)]
            tnext_ap = tf_sb[0:1, ds(iv, 1)]
            nc.vector.tensor_scalar(t_st[:], ones16[:], tprev_ap, None, OP.mult)
            emit_step(tnext_ap, 0)
            with ExitStack() as stk:
                for s in range(1, max_steps):
                    v = nc.values_load(flagi[s - 1][0:1, 0:1],
                                       skip_runtime_bounds_check=True)
                    stk.enter_context(tc.If(v > 0))
                    emit_step(tnext_ap, s)
            off = nc.snap(iv * BL)
            nc.vector.tensor_copy(zsaveA[:, ds(off, BL)], z[:, 0:BL])
            nc.vector.tensor_copy(zsaveB[:, ds(off, BL)], z[:, BL:W2])

    # ================= readout =================
    # ys[c, s*16+b] = (ro_w @ z_s)[c, b] + ro_b[c]; host transposes to [b, s, c]
    with nc.named_scope("readout"), \
         tc.tile_pool(name="pr", bufs=2, space="PSUM") as pr:
        for lo, hi in [(0, RO_SPLIT), (RO_SPLIT, nf * BL)]:
            w = hi - lo
            rop = pr.tile([COUT, RO_SPLIT], F32, tag="rop")
            MM(rop[:, 0:w], sb["roT"][:, 0:COUT], zsaveA[:, lo:hi],
               start=True, stop=False)
            MM(rop[:, 0:w], sb["roT"][:, COUT:2 * COUT], zsaveB[:, lo:hi],
               start=False, stop=False)
            MM(rop[:, 0:w], sb["robr"][:], onesw[0:1, 0:w], start=False, stop=True)
            nc.vector.tensor_copy(ys_sb[:, lo:hi], rop[:, 0:w])
    nc.sync.dma_start(out_d[:], ys_sb[:])

    ctx.close()
    return nc


_CACHE = {}


def _get_program():
    if "nc" not in _CACHE:
        nc = build_program()
        nc.compile()
        _CACHE["nc"] = nc
    return _CACHE["nc"]


def kernel(**inputs):
    nc = _get_program()
    w = _prep_weights(inputs)
    in_maps = []
    for c in range(NCORES):
        m = dict(w)
        m["xT"] = _prep_core_x(inputs["y_past"], c)
        in_maps.append(m)
    res = run_bass_kernel_spmd(nc, in_maps, list(range(NCORES)))
    out = np.stack([
        np.asarray(res.results[c]["out"]).reshape(COUT, NF, BL).transpose(2, 1, 0)
        for c in range(NCORES)])
    return out.reshape(B, NF, COUT).astype(np.float32)


# revision 24
# speedup vs baseline: 3.0073x; 1.0495x over previous
"""GRU-ODE Trainium2 kernel: data-parallel over 8 NeuronCores (16 samples each).

v2: fp16 matmul operands everywhere (1 cycle/row, FWL weight loads, no fp32
double-pass), sigmoid-only GRU phase (tanh synthesized as 2*sigmoid(2x)-1 so a
single activation-table set covers the whole phase), FSAL Dormand-Prince (k7 of
an accepted step is reused as k1 of the next -> 6 MLP evals per RK step),
k-space stage accumulation, batched fp16 readout with host-side transpose.

Phases per core:
  1. GRU encoder: 512 sequential steps, hidden folded [128, 2*16].
  2. Adaptive DOPRI5 ODE solve: 32 intervals x up-to-16 RK steps with
     device-side early exit. Softplus = Ln(exp(u)+1) via the
     natural_log_exp table set; tanh head via Exp + reciprocal.
  3. Readout: two big fp16 matmuls over all 33 save points; host transposes.
"""
import sys
import numpy as np

sys.path.insert(0, "/root/.axon_site/_ro/trn_rl_repo")

import concourse.bass as bass
import concourse.bacc as bacc
import concourse.tile as tile
import concourse.mybir as mybir
from contextlib import ExitStack
from concourse.bass import ds
from concourse.bass_utils import run_bass_kernel_spmd

F32 = mybir.dt.float32
F16 = mybir.dt.float16
I32 = mybir.dt.int32
AF = mybir.ActivationFunctionType
OP = mybir.AluOpType

B, TIN, NF = 128, 512, 33
CIN, H, COUT, WIDTH = 64, 256, 64, 128
MAX_STEPS = 16
RTOL, ATOL = 1e-3, 1e-6
NCORES = 8
BL = B // NCORES  # 16 samples per core
W2 = 2 * BL       # 32: two hidden halves side by side

# Dormand-Prince 5(4) tableau
A_TAB = {
    2: [0.2],
    3: [3 / 40, 9 / 40],
    4: [44 / 45, -56 / 15, 32 / 9],
    5: [19372 / 6561, -25360 / 2187, 64448 / 6561, -212 / 729],
    6: [9017 / 3168, -355 / 33, 46732 / 5247, 49 / 176, -5103 / 18656],
}
B5_TAB = {1: 35 / 384, 3: 500 / 1113, 4: 125 / 192, 5: -2187 / 6784, 6: 11 / 84}
E_TAB = {1: 71 / 57600, 3: -71 / 16695, 4: 71 / 1920,
         5: -17253 / 339200, 6: 22 / 525, 7: -1 / 40}
SUM_A = {s: float(sum(A_TAB[s])) for s in A_TAB}
SUM_B5 = float(sum(B5_TAB.values()))
SUM_E = float(sum(E_TAB.values()))
# scaled-identity slots: 0 = I, 1..5 = B5 coeffs (j=1,3,4,5,6), 6..11 = E coeffs
SID_B5 = {j: i + 1 for i, j in enumerate([1, 3, 4, 5, 6])}
SID_E = {j: i + 6 for i, j in enumerate([1, 3, 4, 5, 6, 7])}
NSID = 12
RO_SPLIT = 272  # readout column split: 33*16 = 272 + 256 (psum bank limit)


def _prep_weights(inp):
    """Host-side: transform weights into the SBUF layouts the kernel wants."""
    h = lambda a: np.ascontiguousarray(a, dtype=np.float16)
    f = lambda a: np.ascontiguousarray(a, dtype=np.float32)
    wih, whh = np.asarray(inp["gru_wih"]), np.asarray(inp["gru_whh"])
    gb, bn = np.asarray(inp["gru_b"]), np.asarray(inp["gru_bn"])
    w0, b0 = np.asarray(inp["w0"]), np.asarray(inp["b0"])
    w1, b1 = np.asarray(inp["w1"]), np.asarray(inp["b1"])
    w2, b2 = np.asarray(inp["w2"]), np.asarray(inp["b2"])
    row, rob = np.asarray(inp["ro_w"]), np.asarray(inp["ro_b"])
    t = np.asarray(inp["t"])

    sid = np.zeros((128, NSID * 128), np.float32)
    eye = np.eye(128, dtype=np.float32)
    sid[:, 0:128] = eye
    for j, slot in SID_B5.items():
        sid[:, slot * 128:(slot + 1) * 128] = eye * np.float32(B5_TAB[j])
    for j, slot in SID_E.items():
        sid[:, slot * 128:(slot + 1) * 128] = eye * np.float32(E_TAB[j])

    w0T = w0.T  # [256, 128]
    roT = row.T  # [256, 64]
    return {
        "wihT": h(np.concatenate([wih.T, gb[None, :]], axis=0)),  # [65, 768]
        "whhT0": h(whh.T[:128]), "whhT1": h(whh.T[128:]),  # [128, 768]
        "bnr": h(bn[None, :]),  # [1, 256]
        "w0T": h(np.concatenate([w0T[:128], w0T[128:]], axis=1)),  # [128, 256]
        "w1T": h(w1.T),  # [128, 128]
        "w2T": h(w2.T),  # [128, 256]
        "b0c": f(b0[:, None]), "b1c": f(b1[:, None]),  # [128, 1]
        "b2m2": f(-2.0 * b2.reshape(2, 128).T),  # [128, 2]
        "roT": h(np.concatenate([roT[:128], roT[128:]], axis=1)),  # [128, 128]
        "robr": h(rob[None, :]),  # [1, 64]
        "sid": h(sid),  # [128, NSID*128]
        "tf": f(t[TIN:][None, :]),  # [1, NF]
    }


def _prep_core_x(y_past, core):
    """y_past [B, TIN, CIN] -> xT_aug [65, TIN*16] fp16, col = t*16+b."""
    yc = np.asarray(y_past, np.float32)[core * BL:(core + 1) * BL]  # [16,T,64]
    xt = yc.transpose(2, 1, 0).reshape(CIN, -1)  # [64, T*16]
    return np.ascontiguousarray(np.concatenate(
        [xt, np.ones((1, xt.shape[1]), np.float32)], axis=0).astype(np.float16))


def _pin_exp_ln_tables(arch):
    """Make natural_log_exp_and_others the only table set advertising Exp/Ln.

    The act-table-load pass keeps the current set when it suffices, else picks
    the FIRST set containing the function. Exp's first match (exp_and_others)
    lacks Ln and vice versa, so Exp<->Ln chains thrash ACT_TABLE_LOAD (~1.3us
    each). Removing exp/ln from the other sets' membership (contents only --
    set order and ids unchanged) forces the one set that truly has both.
    """
    from concourse.hw_specs import get_activation_tables
    tabs = get_activation_tables(arch)  # functools.cache: mutate in place
    for name, fns in tabs.items():
        if name == "natural_log_exp_and_others":
            continue
        fns.discard(AF.Exp)
        fns.discard(AF.Ln)


def build_program(tin=TIN, nf=NF, max_steps=MAX_STEPS):
    nc = bacc.Bacc("TRN2", target_bir_lowering=False, debug=False)
    _pin_exp_ln_tables(nc.m.arch)
    d = {}
    d["xT"] = nc.dram_tensor("xT", [CIN + 1, tin * BL], F16, kind="ExternalInput")
    d["tf"] = nc.dram_tensor("tf", [1, nf], F32, kind="ExternalInput")
    for nm, shp, dt in [
            ("wihT", [65, 768], F16), ("whhT0", [128, 768], F16),
            ("whhT1", [128, 768], F16), ("bnr", [1, 256], F16),
            ("w0T", [128, 256], F16), ("w1T", [128, 128], F16),
            ("w2T", [128, 256], F16), ("b0c", [128, 1], F32),
            ("b1c", [128, 1], F32), ("b2m2", [128, 2], F32),
            ("roT", [128, 128], F16), ("robr", [1, 64], F16),
            ("sid", [128, NSID * 128], F16)]:
        d[nm] = nc.dram_tensor(nm, shp, dt, kind="ExternalInput")
    out_d = nc.dram_tensor("out", [COUT, nf * BL], F32, kind="ExternalOutput")

    ctx = ExitStack()
    tc = ctx.enter_context(tile.TileContext(nc))
    wp = ctx.enter_context(tc.tile_pool(name="w", bufs=1))
    sp = ctx.enter_context(tc.tile_pool(name="s", bufs=1))

    # ---- load weights & inputs ----
    sb = {}
    for nm in ["wihT", "whhT0", "whhT1", "bnr", "w0T", "w1T", "w2T", "b0c",
               "b1c", "b2m2", "roT", "robr", "sid", "tf"]:
        sb[nm] = wp.tile(list(d[nm].shape), d[nm].dtype, tag=nm, name=nm)
        nc.sync.dma_start(sb[nm][:], d[nm][:])
    xT = wp.tile([CIN + 1, tin * BL], F16, tag="xT")
    nchunk = 4
    cw = tin * BL // nchunk
    for k in range(nchunk):
        nc.sync.dma_start(xT[:, k * cw:(k + 1) * cw], d["xT"][:, k * cw:(k + 1) * cw])

    ones16 = wp.tile([1, BL], F32, tag="ones16")       # f32 lane constant
    ones16f = wp.tile([1, BL], F16, tag="ones16f")     # f16 bias-matmul rhs
    onesrf = wp.tile([1, 128], F16, tag="onesrf")      # f16 broadcast lhsT
    onescf = wp.tile([128, 1], F16, tag="onescf")      # f16 reduce lhsT
    onesw = wp.tile([1, RO_SPLIT], F16, tag="onesw")   # readout bias rhs
    eps24 = wp.tile([1, 1], F32, tag="eps24", name="eps24")
    nc.vector.memset(eps24[:], 1e-24)
    nc.vector.memset(ones16[:], 1.0)
    nc.vector.memset(ones16f[:], 1.0)
    nc.vector.memset(onesrf[:], 1.0)
    nc.vector.memset(onescf[:], 1.0)
    nc.vector.memset(onesw[:], 1.0)

    # ---- state tiles (fixed addresses; live across dynamic control flow) ----
    z = sp.tile([128, W2], F16, tag="z")          # folded [hidden-half | sample]
    fs = sp.tile([128, W2], F16, tag="fs")        # FSAL: 2*sigmoid(2*v(z)) = f(z)+1
    t_st = sp.tile([1, BL], F32, tag="t_st")
    dt_st = sp.tile([1, BL], F32, tag="dt_st")
    zsaveA = sp.tile([128, nf * BL], F16, tag="zsaveA")
    zsaveB = sp.tile([128, nf * BL], F16, tag="zsaveB")
    ys_sb = sp.tile([COUT, nf * BL], F32, tag="ys")

    MM = nc.tensor.matmul

    # ================= GRU phase =================
    with nc.named_scope("gru"), \
         tc.tile_pool(name="pg", bufs=1, space="PSUM") as pg, \
         tc.tile_pool(name="gs", bufs=1) as gs:
        # separate tiles (= separate PSUM banks) so sigmoid(r) doesn't wait
        # on the update-gate matmuls (dep tracking is per tile)
        GR = pg.tile([128, W2], F32, tag="GR")       # [ra | rb]
        GU = pg.tile([128, W2], F32, tag="GU")       # [ua | ub]
        PN = pg.tile([128, W2], F32, tag="PN")       # [hn_a | hn_b] (incl bn)
        PI = pg.tile([128, W2], F32, tag="PI")       # [inn_a | inn_b]
        rz = gs.tile([128, 4 * BL], F16, tag="rz")
        q3a = gs.tile([128, W2], F16, tag="q3a")
        q3c = gs.tile([128, W2], F16, tag="q3c")
        s2 = gs.tile([128, W2], F16, tag="s2")
        omz = gs.tile([128, W2], F16, tag="omz")
        zh = gs.tile([128, W2], F16, tag="zh")
        wsum = gs.tile([128, W2], F16, tag="wsum")
        sn = gs.tile([128, W2], F16, tag="sn")
        nc.vector.memset(z[:], 0.0)

        for t in range(tin):
            xs = xT[:, t * BL:(t + 1) * BL]
            za, zb = z[:, 0:BL], z[:, BL:W2]
            # inn (x-only, own tile): runs while step t-1's tail finishes.
            # PSUM accumulation groups within a tile must be sequential
            # (zero-region constraint), so each 16-col group completes
            # before the next one starts.
            MM(PI[:, 0:BL], sb["wihT"][:, 512:640], xs, start=True, stop=True)
            MM(PI[:, BL:W2], sb["wihT"][:, 640:768], xs, start=True, stop=True)
            # r gate first (its sigmoid gates the longest chain)
            MM(GR[:, 0:16], sb["wihT"][:, 0:128], xs, start=True, stop=False)
            MM(GR[:, 0:16], sb["whhT0"][:, 0:128], za, start=False, stop=False)
            MM(GR[:, 0:16], sb["whhT1"][:, 0:128], zb, start=False, stop=True)
            MM(GR[:, 16:32], sb["wihT"][:, 128:256], xs, start=True, stop=False)
            MM(GR[:, 16:32], sb["whhT0"][:, 128:256], za, start=False, stop=False)
            MM(GR[:, 16:32], sb["whhT1"][:, 128:256], zb, start=False, stop=True)
            # n-gate hidden part next (feeds q3 right after sigmoid(r));
            # bn folded in via per-partition bias rows
            MM(PN[:, 0:16], sb["bnr"][0:1, 0:128], ones16f[:], start=True, stop=False)
            MM(PN[:, 0:16], sb["whhT0"][:, 512:640], za, start=False, stop=False)
            MM(PN[:, 0:16], sb["whhT1"][:, 512:640], zb, start=False, stop=True)
            MM(PN[:, 16:32], sb["bnr"][0:1, 128:256], ones16f[:], start=True, stop=False)
            MM(PN[:, 16:32], sb["whhT0"][:, 640:768], za, start=False, stop=False)
            MM(PN[:, 16:32], sb["whhT1"][:, 640:768], zb, start=False, stop=True)
            # update gate last
            MM(GU[:, 0:16], sb["wihT"][:, 256:384], xs, start=True, stop=False)
            MM(GU[:, 0:16], sb["whhT0"][:, 256:384], za, start=False, stop=False)
            MM(GU[:, 0:16], sb["whhT1"][:, 256:384], zb, start=False, stop=True)
            MM(GU[:, 16:32], sb["wihT"][:, 384:512], xs, start=True, stop=False)
            MM(GU[:, 16:32], sb["whhT0"][:, 384:512], za, start=False, stop=False)
            MM(GU[:, 16:32], sb["whhT1"][:, 384:512], zb, start=False, stop=True)

            nc.scalar.activation(rz[:, 0:W2], GR[:], AF.Sigmoid)
            nc.scalar.activation(rz[:, W2:2 * W2], GU[:], AF.Sigmoid)
            nc.vector.tensor_tensor(q3a[:], PN[:], rz[:, 0:W2], OP.mult)
            nc.vector.tensor_tensor(q3c[:], q3a[:], PI[:], OP.add)
            # n = tanh(q3) = 2*sigmoid(2*q3) - 1; z' = 2s*(1-u) + (u*z - (1-u))
            nc.scalar.activation(s2[:], q3c[:], AF.Sigmoid, scale=2.0)
            nc.vector.tensor_scalar(omz[:], rz[:, W2:2 * W2], -1.0, 1.0, OP.mult, OP.add)
            nc.vector.tensor_tensor(zh[:], rz[:, W2:2 * W2], z[:], OP.mult)
            nc.vector.tensor_tensor(wsum[:], zh[:], omz[:], OP.subtract)
            nc.vector.scalar_tensor_tensor(sn[:], s2[:], 2.0, omz[:], OP.mult, OP.mult)
            nc.vector.tensor_tensor(z[:], sn[:], wsum[:], OP.add)

    nc.vector.tensor_copy(zsaveA[:, 0:BL], z[:, 0:BL])
    nc.vector.tensor_copy(zsaveB[:, 0:BL], z[:, BL:W2])

    # ================= ODE phase =================
    with nc.named_scope("ode"), \
         tc.tile_pool(name="po", bufs=1, space="PSUM") as po, \
         tc.tile_pool(name="osb", bufs=1) as osb:
        Pd = po.tile([128, W2], F32, tag="Pd")    # dt broadcast
        Pu = po.tile([128, BL], F32, tag="Pu")    # MLP pre-activations
        Pe = po.tile([128, BL], F32, tag="Pe")    # exp intermediates
        P4 = po.tile([128, W2], F32, tag="P4")    # head pre-activation
        P0 = po.tile([128, W2], F32, tag="P0")    # y5
        P1 = po.tile([128, W2], F32, tag="P1")    # err
        P2 = po.tile([1, W2], F32, tag="P2")      # msq partial
        Pa = po.tile([128, W2], F32, tag="Pa")    # accept broadcast

        dtb2 = osb.tile([128, W2], F16, tag="dtb2")
        dt2f = osb.tile([1, W2], F16, tag="dt2f")
        sy = osb.tile([1, W2], F16, tag="sy")
        se = osb.tile([1, W2], F16, tag="se")
        h0 = osb.tile([128, BL], F16, tag="h0")
        h1 = osb.tile([128, BL], F16, tag="h1")
        ed = osb.tile([128, W2], F32, tag="ed")
        dd = osb.tile([128, W2], F32, tag="dd")
        rcd = osb.tile([128, W2], F32, tag="rcd")
        kk = {j: osb.tile([128, W2], F16, tag=f"kk{j}", name=f"kk{j}")
              for j in range(1, 8)}
        zacc = {s: osb.tile([128, W2], F16, tag=f"zacc{s}", name=f"zacc{s}")
                for s in range(2, 7)}
        y5sb = osb.tile([128, W2], F16, tag="y5sb")
        fs_c = osb.tile([128, W2], F16, tag="fs_c")
        dz = osb.tile([128, W2], F16, tag="dz")
        zm = osb.tile([128, W2], F16, tag="zm")
        dfs = osb.tile([128, W2], F16, tag="dfs")
        fsm = osb.tile([128, W2], F16, tag="fsm")
        mx1 = osb.tile([128, W2], F32, tag="mx1")
        mx2 = osb.tile([128, W2], F32, tag="mx2")
        scm = osb.tile([128, W2], F32, tag="scm")
        rsc = osb.tile([128, W2], F32, tag="rsc")
        qt = osb.tile([128, W2], F32, tag="qt")
        q2 = osb.tile([128, W2], F16, tag="q2")
        msq32 = osb.tile([1, W2], F32, tag="msq32")
        acc32 = osb.tile([1, W2], F16, tag="acc32")
        L16 = lambda tg: osb.tile([1, BL], F32, tag=tg, name=tg)
        rem, mx, dt_use = L16("rem"), L16("mx"), L16("dt_use")
        nd, done = L16("nd"), L16("done")
        tm, lnm, f0 = L16("tm"), L16("lnm"), L16("f0")
        fac, fac2, le, acc16 = L16("fac"), L16("fac2"), L16("le"), L16("acc16")
        st_t, cand, cand2 = L16("st_t"), L16("cand"), L16("cand2")
        remn, ndn = L16("remn"), L16("ndn")
        doneI = osb.tile([1, BL], I32, tag="doneI")
        flagS = [osb.tile([1, 1], F32, tag=f"flag{s}", name=f"flag{s}")
                 for s in range(max_steps)]
        tmpF, tmpL = L16("tmpF"), L16("tmpL")

        tf_sb = sb["tf"]
        # dt0 = (tf[-1] - tf[0]) * 0.01
        nc.vector.tensor_scalar(tmpF[:], ones16[:], tf_sb[0:1, 0:1], None, OP.mult)
        nc.vector.scalar_tensor_tensor(tmpL[:], ones16[:], tf_sb[0:1, nf - 1:nf],
                                       tmpF[:], OP.mult, OP.subtract)
        nc.vector.tensor_scalar(dt_st[:], tmpL[:], 0.01, None, OP.mult)
        # force the exp/ln table load once, outside the interval loop
        nc.scalar.activation(tmpF[:], ones16[:], AF.Exp)
        nc.scalar.activation(tmpF[:], ones16[:], AF.Ln)

        def emit_mlp(rhs, tail):
            """MLP eval on rhs [128, W2] fp16 -> dd = 0.5+0.5*e^-2(v+b2).
            tail() consumes dd (and rcd = 1/dd when need_recip)."""
            MM(Pu[:], sb["w0T"][:, 0:128], rhs[:, 0:BL], start=True, stop=False)
            MM(Pu[:], sb["w0T"][:, 128:256], rhs[:, BL:W2], start=False, stop=True)
            nc.scalar.activation(Pe[:], Pu[:], AF.Exp, bias=sb["b0c"][:, 0:1])
            nc.scalar.activation(h0[:], Pe[:], AF.Ln, bias=1.0)
            MM(Pu[:], sb["w1T"][:], h0[:], start=True, stop=True)
            nc.scalar.activation(Pe[:], Pu[:], AF.Exp, bias=sb["b1c"][:, 0:1])
            nc.scalar.activation(h1[:], Pe[:], AF.Ln, bias=1.0)
            # head: e^-2(v+b2) with b2 folded into the activation bias
            MM(P4[:, 0:BL], sb["w2T"][:, 0:128], h1[:], start=True, stop=True)
            MM(P4[:, BL:W2], sb["w2T"][:, 128:256], h1[:], start=True, stop=True)
            nc.scalar.activation(ed[:, 0:BL], P4[:, 0:BL], AF.Exp, scale=-2.0,
                                 bias=sb["b2m2"][:, 0:1])
            nc.scalar.activation(ed[:, BL:W2], P4[:, BL:W2], AF.Exp, scale=-2.0,
                                 bias=sb["b2m2"][:, 1:2])
            nc.vector.tensor_scalar(dd[:], ed[:], 0.5, 0.5, OP.mult, OP.add)
            nc.vector.reciprocal_approx_fast(out=rcd[:], in_=dd[:])
            tail()

        def emit_step(tnext_ap, sidx):
            # lane control at step start (all f32 [1,16])
            nc.vector.tensor_scalar(rem[:], t_st[:], -1.0, tnext_ap, OP.mult, OP.add)
            nc.vector.tensor_scalar(mx[:], rem[:], 0.0, None, OP.max)
            nc.vector.tensor_tensor(dt_use[:], dt_st[:], mx[:], OP.min)
            nc.vector.tensor_scalar(nd[:], rem[:], 1e-8, None, OP.is_gt)
            nc.vector.tensor_scalar(done[:], rem[:], 1e-8, None, OP.is_le)
            nc.vector.tensor_copy(dt2f[0:1, 0:BL], dt_use[:])
            nc.vector.tensor_copy(dt2f[0:1, BL:W2], dt_use[:])
            MM(Pd[:], onesrf[:], dt2f[:], start=True, stop=True)
            nc.vector.tensor_copy(dtb2[:], Pd[:])
            nc.vector.tensor_scalar(sy[:], dt2f[:], -SUM_B5, None, OP.mult)
            nc.vector.tensor_scalar(se[:], dt2f[:], -SUM_E, None, OP.mult)
            # FSAL: kk1 = dt * (f(z)+1) = dt * fs, no MLP eval needed
            nc.vector.tensor_tensor(kk[1][:], dtb2[:], fs[:], OP.mult)
            MM(P1[:], sb["sid"][:, SID_E[1] * 128:(SID_E[1] + 1) * 128], kk[1][:],
               start=True, stop=False)
            MM(P0[:], sb["sid"][:, 0:128], z[:], start=True, stop=False)
            MM(P0[:], sb["sid"][:, SID_B5[1] * 128:(SID_B5[1] + 1) * 128], kk[1][:],
               start=False, stop=False)
            for s in range(2, 7):
                nc.vector.scalar_tensor_tensor(zacc[s][:], dtb2[:], -SUM_A[s],
                                               z[:], OP.mult, OP.add)
            for s in range(2, 7):
                nc.vector.scalar_tensor_tensor(zacc[s][:], kk[1][:], A_TAB[s][0],
                                               zacc[s][:], OP.mult, OP.add)

            for j in range(2, 7):
                def tail(j=j):
                    nc.vector.tensor_tensor(kk[j][:], dtb2[:], rcd[:], OP.mult)
                    for s2 in range(j + 1, 7):
                        nc.vector.scalar_tensor_tensor(
                            zacc[s2][:], kk[j][:], A_TAB[s2][j - 1], zacc[s2][:],
                            OP.mult, OP.add)
                    if j in SID_B5:
                        MM(P0[:], sb["sid"][:, SID_B5[j] * 128:(SID_B5[j] + 1) * 128],
                           kk[j][:], start=False, stop=False)
                    if j in SID_E:
                        MM(P1[:], sb["sid"][:, SID_E[j] * 128:(SID_E[j] + 1) * 128],
                           kk[j][:], start=False, stop=False)
                emit_mlp(zacc[j], tail)

            # y5 = I@z + sum B5_j kk_j - SUM_B5*dt
            MM(P0[:], onesrf[:], sy[:], start=False, stop=True)
            nc.vector.tensor_copy(y5sb[:], P0[:])
            # overlap with eval7: scale + dz
            nc.vector.tensor_tensor(mx1[:], z[:], y5sb[:], OP.max)
            nc.vector.tensor_tensor(mx2[:], z[:], y5sb[:], OP.min)
            nc.vector.scalar_tensor_tensor(scm[:], mx2[:], -1.0, mx1[:], OP.mult, OP.max)
            nc.vector.tensor_scalar(scm[:], scm[:], RTOL, ATOL, OP.mult, OP.add)
            nc.vector.reciprocal_approx_fast(out=rsc[:], in_=scm[:])
            nc.vector.tensor_tensor(dz[:], y5sb[:], z[:], OP.subtract)

            def tail7():
                nc.vector.tensor_tensor(kk[7][:], dtb2[:], rcd[:], OP.mult)
                nc.vector.tensor_copy(fs_c[:], rcd[:])
                MM(P1[:], onesrf[:], se[:], start=False, stop=False)
                MM(P1[:], sb["sid"][:, SID_E[7] * 128:(SID_E[7] + 1) * 128],
                   kk[7][:], start=False, stop=True)
            emit_mlp(y5sb, tail7)

            # error norm and controller
            nc.vector.tensor_tensor(qt[:], P1[:], rsc[:], OP.mult)
            nc.vector.tensor_tensor(q2[:], qt[:], qt[:], OP.mult)
            MM(P2[0:1, 0:W2], onescf[:], q2[:], start=True, stop=True)
            nc.vector.tensor_copy(msq32[:], P2[0:1, 0:W2])
            nc.vector.tensor_tensor(tm[:], msq32[0:1, 0:BL], msq32[0:1, BL:W2], OP.add)
            # factor on the scalar engine (overlaps the accept path below)
            nc.scalar.activation(lnm[:], tm[:], AF.Ln, scale=1.0 / 256.0,
                                 bias=eps24[0:1, 0:1])
            nc.scalar.activation(f0[:], lnm[:], AF.Exp, scale=-0.1)
            # accept = (tm <= 256) & notdone; flag path first (gates the If)
            nc.vector.tensor_scalar(le[:], tm[:], 256.0, None, OP.is_le)
            nc.vector.tensor_tensor(acc16[:], le[:], nd[:], OP.mult)
            nc.vector.tensor_tensor(st_t[:], acc16[:], dt_use[:], OP.mult)
            nc.vector.tensor_tensor(t_st[:], t_st[:], st_t[:], OP.add)
            nc.vector.tensor_tensor(remn[:], rem[:], st_t[:], OP.subtract)
            nc.vector.reduce_max(flagS[sidx][:], remn[:], axis=mybir.AxisListType.X)
            nc.vector.tensor_copy(acc32[0:1, 0:BL], acc16[:])
            nc.vector.tensor_copy(acc32[0:1, BL:W2], acc16[:])
            MM(Pa[:], onesrf[:], acc32[:], start=True, stop=True)
            # masked state updates: x += accept * (cand - x)
            nc.vector.tensor_tensor(zm[:], Pa[:], dz[:], OP.mult)
            nc.vector.tensor_tensor(z[:], z[:], zm[:], OP.add)
            nc.vector.tensor_tensor(dfs[:], fs_c[:], fs[:], OP.subtract)
            nc.vector.tensor_tensor(fsm[:], Pa[:], dfs[:], OP.mult)
            nc.vector.tensor_tensor(fs[:], fs[:], fsm[:], OP.add)
            # dt update: clip(0.9*(tm/256)^-0.1, 0.2, 10), frozen for done lanes
            nc.vector.tensor_scalar(fac[:], f0[:], 0.9, 0.2, OP.mult, OP.max)
            nc.vector.tensor_scalar(fac2[:], fac[:], 10.0, None, OP.min)
            nc.vector.tensor_tensor(cand[:], dt_use[:], fac2[:], OP.mult)
            nc.vector.tensor_scalar(cand2[:], cand[:], 1e-6, None, OP.max)
            nc.vector.tensor_copy(doneI[:], done[:])
            nc.vector.copy_predicated(cand2[:], doneI[:], dt_st[:])
            nc.vector.tensor_copy(dt_st[:], cand2[:])

        # initial FSAL eval: fs = 2*sigmoid(2*v(z))
        def tail0():
            nc.vector.tensor_copy(fs[:], rcd[:])
        emit_mlp(z, tail0)

        # If condition: raw f32 bits of max remaining time vs bits(1e-8)
        # (positive-float bit patterns are order-preserving as int32)
        THRESH_BITS = int(np.float32(1e-8).view(np.int32))
        with tc.For_i(1, nf, hint_engines=tuple(mybir.ALL_ENGINES)) as iv:
            tprev_ap = tf_sb[0:1, ds(iv - 1, 1)]
            tnext_ap = tf_sb[0:1, ds(iv, 1)]
            nc.vector.tensor_scalar(t_st[:], ones16[:], tprev_ap, None, OP.mult)
            emit_step(tnext_ap, 0)
            with ExitStack() as stk:
                for s in range(1, max_steps):
                    v = nc.values_load(flagS[s - 1][0:1, 0:1].bitcast(I32),
                                       skip_runtime_bounds_check=True)
                    stk.enter_context(tc.If(v > THRESH_BITS))
                    emit_step(tnext_ap, s)
            off = nc.snap(iv * BL)
            nc.vector.tensor_copy(zsaveA[:, ds(off, BL)], z[:, 0:BL])
            nc.vector.tensor_copy(zsaveB[:, ds(off, BL)], z[:, BL:W2])

    # ================= readout =================
    # ys[c, s*16+b] = (ro_w @ z_s)[c, b] + ro_b[c]; host transposes to [b, s, c]
    with nc.named_scope("readout"), \
         tc.tile_pool(name="pr", bufs=2, space="PSUM") as pr:
        for lo, hi in [(0, RO_SPLIT), (RO_SPLIT, nf * BL)]:
            w = hi - lo
            rop = pr.tile([COUT, RO_SPLIT], F32, tag="rop")
            MM(rop[:, 0:w], sb["roT"][:, 0:COUT], zsaveA[:, lo:hi],
               start=True, stop=False)
            MM(rop[:, 0:w], sb["roT"][:, COUT:2 * COUT], zsaveB[:, lo:hi],
               start=False, stop=False)
            MM(rop[:, 0:w], sb["robr"][:], onesw[0:1, 0:w], start=False, stop=True)
            nc.vector.tensor_copy(ys_sb[:, lo:hi], rop[:, 0:w])
    nc.sync.dma_start(out_d[:], ys_sb[:])

    ctx.close()
    return nc


_CACHE = {}


def _get_program():
    if "nc" not in _CACHE:
        nc = build_program()
        nc.compile()
        _CACHE["nc"] = nc
    return _CACHE["nc"]


def kernel(**inputs):
    nc = _get_program()
    w = _prep_weights(inputs)
    in_maps = []
    for c in range(NCORES):
        m = dict(w)
        m["xT"] = _prep_core_x(inputs["y_past"], c)
        in_maps.append(m)
    res = run_bass_kernel_spmd(nc, in_maps, list(range(NCORES)))
    out = np.stack([
        np.asarray(res.results[c]["out"]).reshape(COUT, NF, BL).transpose(2, 1, 0)
        for c in range(NCORES)])
    return out.reshape(B, NF, COUT).astype(np.float32)
